# revision 58
# baseline (speedup 1.0000x reference)
"""Trainium2 Bass kernel for a pre-LN transformer block (MHA + 3-layer FFN).

Sharding: data-parallel over (batch, query-chunk-pair) -> 8 cores, each
owning 1024 query tokens of one batch row.  To balance causal work, each
core owns a folded pair of 512-token chunks (chunk c and chunk 3-c of the
row), so both cores of a row see the same causal block structure.  K/V are
computed over the full 2048-token context per row (duplicated across the 2
cores sharing a row) - no cross-core collectives.  Context columns are
ordered own-chunks-first; the host-built per-core causal mask carries the
global positions.

On-device layout: all activations are TRANSPOSED, [feature, token],
features tiled 128-per-partition.  Every matmul is out^T = lhsT.T @ rhs
with lhsT = W in natural [in, out] layout, so the convention is maintained
end-to-end with zero on-chip transposes.  Attention scores are computed
transposed ([key, query]); softmax normalization reduces over the
partition (key) axis via a ones-column appended to V, so the denominator
falls out of the same PE accumulation that computes attn @ V.  LayerNorm
reduces over the partition (feature) axis with ones-vector matmuls
(float32r: single-pass fp32) on the PE.

Causal structure (uniform across cores by construction): s-tiles 0-7 are
the own chunks, 8-15 the other two chunks ascending.  Tiles {0-3, 8-11}
are needed by all 1024 queries ("full" units); tiles {4-7, 12-15} only by
the upper 512 queries ("half" units) - 25% of score/softmax/AV work is
statically skipped.
"""

import os

import numpy as np
import ml_dtypes

B, T, C = 4, 2048, 1024
H, D = 16, 64
FF = 4 * C
EPS = 1e-5
P = 128
CC = C // P          # 8 feature chunks
FOC = FF // P        # 32 ff chunks
TQ = 1024            # own (query) tokens per core
TC = 2048            # context tokens per core
NCORES = 8
SCALE = float(C) ** -0.5

FULL_ST = [0, 1, 2, 3, 8, 9, 10, 11]     # key tiles needed by all queries
HALF_ST = [4, 5, 6, 7, 12, 13, 14, 15]   # key tiles needed by queries 512:1024

BF16 = ml_dtypes.bfloat16

_PROG = None         # compiled Bacc program, built once per process
LAST_RESULT = None   # BassKernelResults of the most recent run (for profiling)


def _build_program():
    import concourse.mybir as mybir
    import concourse.tile as tile
    from concourse import bacc

    f32 = mybir.dt.float32
    f16 = mybir.dt.float16
    bf16 = mybir.dt.bfloat16
    AF = mybir.ActivationFunctionType
    ALU = mybir.AluOpType

    nc = bacc.Bacc("TRN2", target_bir_lowering=False, debug=False)

    # ---- DRAM parameters (per-core shapes; all pre-packed on host) ----
    xq = nc.declare_dram_parameter("xq", [P, CC, TQ], f32, isOutput=False)
    xTb = nc.declare_dram_parameter("xTb", [P, CC, TC], bf16, isOutput=False)
    maskT = nc.declare_dram_parameter("maskT", [P, 16, 512], bf16,
                                      isOutput=False)
    wq = nc.declare_dram_parameter("wq", [8, P, CC, P], bf16, isOutput=False)
    wk = nc.declare_dram_parameter("wk", [8, P, CC, P], bf16, isOutput=False)
    wv = nc.declare_dram_parameter("wv", [2, P, CC, 512], bf16, isOutput=False)
    wproj = nc.declare_dram_parameter("wproj", [P, CC, C], bf16, isOutput=False)
    w1 = nc.declare_dram_parameter("w1", [FOC, P, CC, P], bf16, isOutput=False)
    w2 = nc.declare_dram_parameter("w2", [FOC, P, FOC, P], bf16, isOutput=False)
    w3 = nc.declare_dram_parameter("w3", [CC, P, FOC, P], bf16, isOutput=False)
    bproj = nc.declare_dram_parameter("bproj", [P, CC], f32, isOutput=False)
    b1 = nc.declare_dram_parameter("b1", [P, FOC], f32, isOutput=False)
    b2 = nc.declare_dram_parameter("b2", [P, FOC], f32, isOutput=False)
    b3 = nc.declare_dram_parameter("b3", [P, CC], f32, isOutput=False)
    qb = nc.declare_dram_parameter("qb", [P, 8], f32, isOutput=False)
    kb = nc.declare_dram_parameter("kb", [P, 8], f32, isOutput=False)
    outT = nc.declare_dram_parameter("outT", [P, CC, TQ], f32, isOutput=True)

    TSW = 256  # layernorm column-slice width

    def layernorm_T(lp, lpr, lps, fill_xslice, ncols, out,
                    oc_col, ones_row, eps_sb):
        """Feature-axis LN (affine folded into downstream weights/biases on
        the host) of transposed bf16 activations, software-pipelined two
        slices deep: the normalize (PE broadcast + DVE) of slice ts-2 is
        emitted after the stats matmuls of slice ts so the in-order PE
        queue never stalls on the row-stats chain.  x and x^2 are packed
        adjacently per chunk so one bf16 ones-matmul (with 1/C folded into
        the ones value) yields both E[x] and E[x^2].  The reciprocal runs
        on the 128-lane broadcast of std, not the 1-lane row."""
        pends = []

        def emit_norm(p):
            ts, xsqs, mrow, std, sl = p
            ps_mu = lps.tile([P, TSW], f32, tag="ps_mu")
            nc.tensor.matmul(ps_mu[:], ones_row[:], mrow[:, 0:TSW],
                             start=True, stop=True)
            ps_rs = lps.tile([P, TSW], f32, tag="ps_rs")
            nc.tensor.matmul(ps_rs[:], ones_row[:], std[:],
                             start=True, stop=True)
            rsB = lp.tile([P, TSW], f32, tag="ln_rsB")
            nc.vector.reciprocal(rsB[:], ps_rs[:])
            for cc in range(CC):
                t1 = lp.tile([P, TSW], f32, tag="ln_t1")
                nc.vector.tensor_sub(t1[:], xsqs[cc][:, 0, :], ps_mu[:])
                nc.vector.tensor_mul(out[:, cc, sl], t1[:], rsB[:])

        for ts in range(ncols // TSW):
            sl = slice(ts * TSW, (ts + 1) * TSW)
            xsqs = [lp.tile([P, 2, TSW], bf16, tag=f"xsq{cc}", name=f"xsq{cc}")
                    for cc in range(CC)]
            fill_xslice(xsqs, ts, sl)
            ps_st = lps.tile([1, 2 * TSW], f32, tag="ps_st")
            for cc in range(CC):
                if cc < 4:
                    nc.scalar.square(xsqs[cc][:, 1, :], xsqs[cc][:, 0, :])
                else:
                    nc.vector.tensor_mul(xsqs[cc][:, 1, :], xsqs[cc][:, 0, :],
                                         xsqs[cc][:, 0, :])
                nc.tensor.matmul(ps_st[:], oc_col[:],
                                 xsqs[cc].rearrange("p a b -> p (a b)"),
                                 start=(cc == 0), stop=(cc == CC - 1))
            mrow = lpr.tile([1, 2 * TSW], f32, tag="ln_mrow")
            nc.vector.tensor_copy(mrow[:], ps_st[:])
            msq = lpr.tile([1, TSW], f32, tag="ln_msq")
            nc.vector.tensor_mul(msq[:], mrow[:, 0:TSW], mrow[:, 0:TSW])
            nc.vector.tensor_sub(mrow[:, TSW:2 * TSW],
                                 mrow[:, TSW:2 * TSW], msq[:])
            std = lpr.tile([1, TSW], f32, tag="ln_std")
            nc.scalar.activation(std[:], mrow[:, TSW:2 * TSW], AF.Sqrt,
                                 bias=eps_sb[:])
            pends.append((ts, xsqs, mrow, std, sl))
            if len(pends) > 2:
                emit_norm(pends.pop(0))
        for p in pends:
            emit_norm(p)

    with tile.TileContext(nc) as tc:
        with tc.tile_pool(name="const", bufs=1) as cp:
            oc_col = cp.tile([P, 1], bf16)
            nc.vector.memset(oc_col[:], 1.0 / C)
            ones_row = cp.tile([1, P], f32)
            nc.vector.memset(ones_row[:], 1.0)
            eps_sb = cp.tile([1, 1], f32)
            nc.vector.memset(eps_sb[:], EPS)
            ones_row_h = cp.tile([1, 64], f16)
            nc.vector.memset(ones_row_h[:], 1.0)

            qb_sb = cp.tile([P, 8], f32, tag="qb")
            nc.sync.dma_start(qb_sb[:], qb[:])
            kb_sb = cp.tile([P, 8], f32, tag="kb")
            nc.sync.dma_start(kb_sb[:], kb[:])
            bproj_sb = cp.tile([P, CC], f32, tag="bproj")
            nc.sync.dma_start(bproj_sb[:], bproj[:])
            b1_sb = cp.tile([P, FOC], f32, tag="b1")
            nc.sync.dma_start(b1_sb[:], b1[:])
            b2_sb = cp.tile([P, FOC], f32, tag="b2")
            nc.sync.dma_start(b2_sb[:], b2[:])
            b3_sb = cp.tile([P, CC], f32, tag="b3")
            nc.sync.dma_start(b3_sb[:], b3[:])

            with tc.tile_pool(name="x1", bufs=1) as p_x1:
                x1T = p_x1.tile([P, CC, TQ], f32, tag="x1T")

                # ======== attention residual branch ========
                with tc.tile_pool(name="ao", bufs=1) as p_ao:
                    attnT = p_ao.tile([P, CC, TQ], bf16, tag="attnT")

                    with tc.tile_pool(name="qkv", bufs=1) as qp:
                        kT = qp.tile([P, 8, TC], bf16, tag="kT")   # head pairs
                        qT = qp.tile([P, 8, TQ], bf16, tag="qT")
                        V_aug = qp.tile([P, 16, H, 65], bf16, tag="V_aug")
                        nc.vector.memset(V_aug[:], 1.0)  # ones col @ index 64

                        with tc.tile_pool(name="h", bufs=1) as php:
                            hT = php.tile([P, CC, TC], bf16, tag="hT")

                            # ---------------- LN1 over full context --------
                            with (
                                tc.tile_pool(name="ln1p", bufs=2) as lp,
                                tc.tile_pool(name="ln1r", bufs=2) as lpr,
                                tc.tile_pool(name="ln1ps", bufs=2,
                                             space="PSUM") as lps,
                            ):
                                def fill_x1slice(xsqs, ts, sl):
                                    for cc in range(CC):
                                        nc.sync.dma_start(xsqs[cc][:, 0, :],
                                                          xTb[:, cc, sl])
                                layernorm_T(lp, lpr, lps, fill_x1slice, TC,
                                            hT, oc_col, ones_row, eps_sb)

                            # ------- merged QKV + attention -------
                            # QKV work is cut into small self-contained
                            # matmul chains.  Pairs 0-1 and the first half of
                            # V run up front; the rest is drip-fed between
                            # attention units so the in-order PE queue never
                            # drains (keeps the HAM clock warm).
                            half_pairs = [(HALF_ST[i], HALF_ST[i + 1])
                                          for i in range(0, 8, 2)]
                            with (
                                tc.tile_pool(name="qkvw", bufs=2) as qwp,
                                tc.tile_pool(name="qkvps", bufs=3,
                                             space="PSUM") as qps,
                                tc.tile_pool(name="attn", bufs=2) as atp,
                                tc.tile_pool(name="attnm", bufs=1) as amp,
                                tc.tile_pool(name="attnps", bufs=1,
                                             space="PSUM") as aps,
                                tc.tile_pool(name="attnps1", bufs=1,
                                             space="PSUM") as aps1,
                            ):
                                mask_sb = amp.tile([P, 16, 512], bf16,
                                                   tag="mask")
                                nc.sync.dma_start(mask_sb[:], maskT[:])

                                def kq_chain(wsrc, bias_sb, dest_kT, pair, js):
                                    sl = slice(js * 512, (js + 1) * 512)

                                    def go():
                                        wt = qwp.tile([P, CC, P], bf16,
                                                      tag="wkq", name="wkq")
                                        nc.sync.dma_start(wt[:], wsrc[pair])
                                        ps_f = qps.tile([P, 512], f32,
                                                        tag="ps_f", name="ps_f")
                                        for cc in range(CC):
                                            nc.tensor.matmul(
                                                ps_f[:], wt[:, cc, :],
                                                hT[:, cc, sl],
                                                start=(cc == 0),
                                                stop=(cc == CC - 1))
                                        nc.vector.tensor_scalar_add(
                                            dest_kT[:, pair, sl], ps_f[:],
                                            bias_sb[:, pair:pair + 1])
                                    return go

                                def v_chain(js2, sc, vh):
                                    def go():
                                        wt = qwp.tile([P, CC, 128], bf16,
                                                      tag="wv", name="wv")
                                        nc.sync.dma_start(
                                            wt[:],
                                            wv[js2][:, :,
                                                    vh * 128:(vh + 1) * 128])
                                        ps_f = qps.tile([P, 512], f32,
                                                        tag="ps_f", name="ps_f")
                                        for cc in range(CC):
                                            nc.tensor.matmul(
                                                ps_f[:, 0:128],
                                                hT[:, cc, sc * P:(sc + 1) * P],
                                                wt[:, cc, :],
                                                start=(cc == 0),
                                                stop=(cc == CC - 1))
                                        for hh in range(2):
                                            hd = js2 * 8 + vh * 2 + hh
                                            nc.vector.tensor_copy(
                                                V_aug[:, sc, hd, 0:64],
                                                ps_f[:, hh * 64:(hh + 1) * 64])
                                    return go

                                def pair_chains(pair):
                                    return ([kq_chain(wk, kb_sb, kT, pair, js)
                                             for js in range(4)]
                                            + [kq_chain(wq, qb_sb, qT, pair, js)
                                               for js in range(2)])

                                # up-front: pairs 0-1 and V half 0
                                for ch in pair_chains(0) + pair_chains(1):
                                    ch()
                                for sc in range(16):
                                    for vh in range(4):
                                        v_chain(0, sc, vh)()

                                # drip-fed fillers with readiness deadlines:
                                # (chains-remaining ceiling before head h)
                                fill = []
                                fill += pair_chains(2) + pair_chains(3)
                                fill += pair_chains(4)
                                for sc in range(16):
                                    for vh in range(4):
                                        fill.append(v_chain(1, sc, vh))
                                fill += (pair_chains(5) + pair_chains(6)
                                         + pair_chains(7))
                                n_fill = len(fill)   # 100
                                deadline = {4: n_fill - 6, 6: n_fill - 12,
                                            8: 18, 10: 12, 12: 6, 14: 0}
                                fill = list(reversed(fill))  # pop() from end

                                def drip(n):
                                    for _ in range(n):
                                        if fill:
                                            fill.pop()()

                                pend = None

                                def emit_norm(p):
                                    dn, avf, h = p
                                    pair, off = h // 2, (h % 2) * 64
                                    for js in range(2):
                                        sl = slice(js * 512, (js + 1) * 512)
                                        ps_bc = aps1.tile([64, 512], f32,
                                                          tag="ps_bc",
                                                          name="ps_bc")
                                        nc.tensor.matmul(ps_bc[:],
                                                         ones_row_h[:],
                                                         dn[:, sl],
                                                         start=True, stop=True)
                                        rcB = atp.tile([64, 512], f32,
                                                       tag="rcB", name="rcB")
                                        nc.vector.reciprocal_approx_fast(
                                            rcB[:], ps_bc[:])
                                        nc.vector.tensor_mul(
                                            attnT[off:off + 64, pair, sl],
                                            avf[:, sl], rcB[:])

                                for h in range(H):
                                    pair, off = h // 2, (h % 2) * 64
                                    while len(fill) > deadline.get(h, n_fill):
                                        fill.pop()()
                                    ps_av = aps1.tile([65, TQ], f32,
                                                      tag="ps_av")
                                    # 8 full units: all 1024 queries vs one
                                    # key tile
                                    for ui, st in enumerate(FULL_ST):
                                        if ui == 2 and pend is not None:
                                            emit_norm(pend)
                                            pend = None
                                        if h < 8 or ui % 2 == 1:
                                            drip(1)
                                        ps_s = aps.tile([P, TQ], f32,
                                                        tag="ps_s")
                                        for js in range(2):
                                            sl = slice(js * 512, (js + 1) * 512)
                                            nc.tensor.matmul(
                                                ps_s[:, sl],
                                                kT[off:off + 64, pair,
                                                   st * P:(st + 1) * P],
                                                qT[off:off + 64, pair, sl],
                                                start=True, stop=True)
                                        ex = atp.tile([P, TQ], bf16, tag="ex")
                                        nc.scalar.activation(ex[:], ps_s[:],
                                                             AF.Exp,
                                                             scale=SCALE)
                                        # only queries 0:512 can be masked
                                        # here; the upper half is statically
                                        # all-ones
                                        nc.vector.tensor_mul(
                                            ex[:, 0:512], ex[:, 0:512],
                                            mask_sb[:, st, :])
                                        for js in range(2):
                                            sl = slice(js * 512, (js + 1) * 512)
                                            nc.tensor.matmul(
                                                ps_av[:, sl],
                                                V_aug[:, st, h, :],
                                                ex[:, sl],
                                                start=(ui == 0),
                                                stop=(ui == 7 and js == 0))
                                    # 4 half-unit pairs: queries 512:1024 vs
                                    # two key tiles, sharing one psum/exp
                                    for pi, (stA, stB) in enumerate(half_pairs):
                                        if h < 8 or pi % 2 == 0:
                                            drip(1)
                                        ps_s = aps.tile([P, TQ], f32,
                                                        tag="ps_s")
                                        for js, st in ((0, stA), (1, stB)):
                                            sl = slice(js * 512, (js + 1) * 512)
                                            nc.tensor.matmul(
                                                ps_s[:, sl],
                                                kT[off:off + 64, pair,
                                                   st * P:(st + 1) * P],
                                                qT[off:off + 64, pair,
                                                   512:1024],
                                                start=True, stop=True)
                                        ex = atp.tile([P, TQ], bf16, tag="ex")
                                        nc.scalar.activation(ex[:], ps_s[:],
                                                             AF.Exp,
                                                             scale=SCALE)
                                        for js, st in ((0, stA), (1, stB)):
                                            sl = slice(js * 512, (js + 1) * 512)
                                            nc.vector.tensor_mul(
                                                ex[:, sl], ex[:, sl],
                                                mask_sb[:, st, :])
                                            nc.tensor.matmul(
                                                ps_av[:, 512:1024],
                                                V_aug[:, st, h, :], ex[:, sl],
                                                start=False,
                                                stop=(pi == 3 and js == 1))
                                    dn = atp.tile([1, TQ], f16, tag="dn")
                                    nc.vector.tensor_copy(dn[:],
                                                          ps_av[64:65, :])
                                    avf = atp.tile([64, TQ], bf16, tag="avf")
                                    nc.vector.tensor_copy(avf[:],
                                                          ps_av[0:64, :])
                                    pend = (dn, avf, h)
                                drip(len(fill))
                                emit_norm(pend)

                    # ---------------- out-projection + residual ------------
                    with (
                        tc.tile_pool(name="proj", bufs=3) as pp,
                        tc.tile_pool(name="projw", bufs=1) as pwp,
                        tc.tile_pool(name="projps", bufs=4, space="PSUM") as pps,
                    ):
                        wp_sb = pwp.tile([P, CC, C], bf16, tag="wproj")
                        nc.sync.dma_start(wp_sb[:], wproj[:])
                        xown = pwp.tile([P, CC, TQ], f32, tag="xown")
                        nc.sync.dma_start(xown[:], xq[:])
                        for co in range(CC):
                            for js in range(2):
                                sl = slice(js * 512, (js + 1) * 512)
                                ps_p = pps.tile([P, 512], f32, tag="ps_p")
                                for ci in range(CC):
                                    nc.tensor.matmul(
                                        ps_p[:],
                                        wp_sb[:, ci, co * P:(co + 1) * P],
                                        attnT[:, ci, sl],
                                        start=(ci == 0), stop=(ci == CC - 1))
                                t1 = pp.tile([P, 512], f32, tag="pj_t1")
                                nc.vector.tensor_scalar_add(
                                    t1[:], ps_p[:], bproj_sb[:, co:co + 1])
                                nc.vector.tensor_add(x1T[:, co, sl], t1[:],
                                                     xown[:, co, sl])

                # ======== FFN residual branch ========
                with tc.tile_pool(name="h2", bufs=1) as p_h2:
                    h2T = p_h2.tile([P, FOC, TQ], bf16, tag="h2T")

                    with tc.tile_pool(name="f1", bufs=1) as fp_in:
                        h2in = fp_in.tile([P, CC, TQ], bf16, tag="h2in")
                        # -------- LN2 --------
                        with (
                            tc.tile_pool(name="ln2p", bufs=2) as lp2,
                            tc.tile_pool(name="ln2r", bufs=2) as lpr2,
                            tc.tile_pool(name="ln2ps", bufs=2,
                                         space="PSUM") as lps2,
                        ):
                            def fill_x2slice(xsqs, ts, sl):
                                for cc in range(CC):
                                    nc.vector.tensor_copy(xsqs[cc][:, 0, :],
                                                          x1T[:, cc, sl])
                            layernorm_T(lp2, lpr2, lps2, fill_x2slice, TQ,
                                        h2in, oc_col, ones_row, eps_sb)
                        # -------- Y1 = relu(h2in @ W1 + b1) --------
                        with (
                            tc.tile_pool(name="f1w", bufs=2) as fwp1,
                            tc.tile_pool(name="f1ps", bufs=4,
                                         space="PSUM") as fps1,
                        ):
                            for fo in range(FOC):
                                w1t = fwp1.tile([P, CC, P], bf16, tag="w1t")
                                nc.sync.dma_start(w1t[:], w1[fo])
                                for js in range(2):
                                    sl = slice(js * 512, (js + 1) * 512)
                                    ps1 = fps1.tile([P, 512], f32, tag="ps1")
                                    for cc in range(CC):
                                        nc.tensor.matmul(
                                            ps1[:], w1t[:, cc, :],
                                            h2in[:, cc, sl],
                                            start=(cc == 0), stop=(cc == CC - 1))
                                    nc.scalar.activation(
                                        h2T[:, fo, sl], ps1[:], AF.Relu,
                                        bias=b1_sb[:, fo:fo + 1])

                    # -------- Y2 = relu(h2T @ W2 + b2) --------
                    with tc.tile_pool(name="h3", bufs=1) as p_h3:
                        h3T = p_h3.tile([P, FOC, TQ], bf16, tag="h3T")
                        with (
                            tc.tile_pool(name="f2w", bufs=3) as fwp2,
                            tc.tile_pool(name="f2ps", bufs=4,
                                         space="PSUM") as fps2,
                        ):
                            for fo in range(FOC):
                                w2t = fwp2.tile([P, FOC, P], bf16, tag="w2t")
                                nc.sync.dma_start(w2t[:], w2[fo])
                                for js in range(2):
                                    sl = slice(js * 512, (js + 1) * 512)
                                    ps2 = fps2.tile([P, 512], f32, tag="ps2")
                                    for fi in range(FOC):
                                        nc.tensor.matmul(
                                            ps2[:], w2t[:, fi, :],
                                            h2T[:, fi, sl],
                                            start=(fi == 0),
                                            stop=(fi == FOC - 1))
                                    nc.scalar.activation(
                                        h3T[:, fo, sl], ps2[:], AF.Relu,
                                        bias=b2_sb[:, fo:fo + 1])

                        # -------- Y3 + bias + residual -> out --------
                        with (
                            tc.tile_pool(name="f3", bufs=3) as fp3,
                            tc.tile_pool(name="f3w", bufs=2) as fwp3,
                            tc.tile_pool(name="f3ps", bufs=4,
                                         space="PSUM") as fps3,
                        ):
                            for co in range(CC):
                                w3t = fwp3.tile([P, FOC, P], bf16, tag="w3t")
                                nc.sync.dma_start(w3t[:], w3[co])
                                for js in range(2):
                                    sl = slice(js * 512, (js + 1) * 512)
                                    ps3 = fps3.tile([P, 512], f32, tag="ps3")
                                    for fi in range(FOC):
                                        nc.tensor.matmul(
                                            ps3[:], w3t[:, fi, :],
                                            h3T[:, fi, sl],
                                            start=(fi == 0),
                                            stop=(fi == FOC - 1))
                                    ot = fp3.tile([P, 512], f32, tag="ot")
                                    nc.vector.tensor_scalar_add(
                                        ot[:], ps3[:], b3_sb[:, co:co + 1])
                                    nc.vector.tensor_add(ot[:], ot[:],
                                                         x1T[:, co, sl])
                                    nc.sync.dma_start(outT[:, co, sl], ot[:])

    nc.compile()
    return nc


def _pack_vec(v, nchunks):
    # [nchunks*P] -> [P, nchunks]
    return np.ascontiguousarray(np.asarray(v, dtype=np.float32).reshape(nchunks, P).T)


def _pack_w(w, in_chunks, out_dim):
    # [in, out] -> [P, in_chunks, out]  (lhsT tiles, fully resident)
    w = np.asarray(w).astype(BF16)
    return np.ascontiguousarray(w.reshape(in_chunks, P, out_dim).transpose(1, 0, 2))


def _pack_w_stream(w, in_chunks, out_chunks, out_w=P):
    # [in, out] -> [out_chunks, P, in_chunks, out_w]  (streamed lhsT tiles)
    w = np.asarray(w).astype(BF16)
    return np.ascontiguousarray(
        w.reshape(in_chunks, P, out_chunks, out_w).transpose(2, 1, 0, 3))


def _core_chunks(core):
    """Global 512-token chunk ids (of the batch row) in this core's context
    column order: own folded pair first, then the other two ascending."""
    c = core % 2
    own = [c, 3 - c]
    others = sorted(set(range(4)) - set(own))
    return own, others


def kernel(x, Wq, Wk, Wv, Wproj, bproj, W1, b1, W2, b2, W3, b3,
           ln1_g, ln1_b, ln2_g, ln2_b):
    global _PROG, LAST_RESULT
    from concourse.bass_utils import run_bass_kernel_spmd

    if _PROG is None:
        _PROG = _build_program()

    x = np.asarray(x, dtype=np.float32)

    # Fold the LN affine transforms into the consuming weights/biases:
    #   h_full = g * h_raw + b  with  h_raw = (x - mu) * rstd
    # Q/K/V/W1 rows get scaled by g; the b contribution becomes an additive
    # bias: per-d for Q/K (applied at PSUM eviction), constant-through-
    # softmax for V (folded into bproj via Wproj), and per-ff for W1.
    g1 = np.asarray(ln1_g, dtype=np.float32)[:, None]
    b1v = np.asarray(ln1_b, dtype=np.float32)
    g2 = np.asarray(ln2_g, dtype=np.float32)[:, None]
    b2v = np.asarray(ln2_b, dtype=np.float32)
    Wq2 = np.asarray(Wq, dtype=np.float32).transpose(1, 0, 2).reshape(C, C)
    Wk2 = np.asarray(Wk, dtype=np.float32).transpose(1, 0, 2).reshape(C, C)
    Wv2 = np.asarray(Wv, dtype=np.float32).transpose(1, 0, 2).reshape(C, C)
    W1f = np.asarray(W1, dtype=np.float32)
    vb = b1v @ Wv2                                   # [C], per-(h,d) V bias
    bproj_f = np.asarray(bproj, dtype=np.float32) + vb @ np.asarray(
        Wproj, dtype=np.float32)
    b1_f = np.asarray(b1, dtype=np.float32) + b2v @ W1f

    common = {
        "wq": _pack_w_stream(g1 * Wq2, CC, 8),
        "wk": _pack_w_stream(g1 * Wk2, CC, 8),
        "wv": _pack_w_stream(g1 * Wv2, CC, 2, 512),
        "wproj": _pack_w(Wproj, CC, C),
        "w1": _pack_w_stream(g2 * W1f, CC, FOC),
        "w2": _pack_w_stream(W2, FOC, FOC),
        "w3": _pack_w_stream(W3, FOC, CC),
        "qb": _pack_vec(b1v @ Wq2, 8),
        "kb": _pack_vec(b1v @ Wk2, 8),
        "bproj": _pack_vec(bproj_f, CC),
        "b1": _pack_vec(b1_f, FOC), "b2": _pack_vec(b2, FOC),
        "b3": _pack_vec(b3, CC),
    }

    ar512 = np.arange(512)
    in_maps = []
    for core in range(NCORES):
        b = core // 2
        own, others = _core_chunks(core)
        order = own + others
        xcat = np.concatenate([x[b, ch * 512:(ch + 1) * 512] for ch in order],
                              axis=0)                       # [2048, C]
        xT_p = np.ascontiguousarray(
            xcat.T.reshape(CC, P, TC).transpose(1, 0, 2))   # [P, CC, TC]
        kpos = np.concatenate([ch * 512 + ar512 for ch in order])
        qpos = np.concatenate([ch * 512 + ar512 for ch in own])
        mask = (kpos[:, None] <= qpos[None, :]).astype(BF16)      # [TC, TQ]
        mask16 = mask.reshape(16, P, TQ)
        # each key tile only ever sees one query half: lower (0:512) for
        # "full" units, upper (512:1024) for "half" units
        mask_p = np.empty((P, 16, 512), dtype=BF16)
        for st in range(16):
            q0 = 0 if st in FULL_ST else 512
            mask_p[:, st, :] = mask16[st, :, q0:q0 + 512]
        in_maps.append({**common,
                        "xq": np.ascontiguousarray(xT_p[:, :, 0:TQ]),
                        "xTb": xT_p.astype(BF16),
                        "maskT": mask_p})

    trace = bool(os.environ.get("BASS_TRACE"))
    res = run_bass_kernel_spmd(_PROG, in_maps, core_ids=list(range(NCORES)),
                               trace=trace)
    LAST_RESULT = res

    out = np.empty((B, T, C), dtype=np.float32)
    for core in range(NCORES):
        b = core // 2
        own, _ = _core_chunks(core)
        oT = res.results[core]["outT"]                      # [P, CC, TQ]
        o2 = oT.transpose(2, 1, 0).reshape(TQ, C)           # [q, C]
        for i, ch in enumerate(own):
            out[b, ch * 512:(ch + 1) * 512] = o2[i * 512:(i + 1) * 512]
    return out


# revision 64
# speedup vs baseline: 1.0058x; 1.0058x over previous
"""Trainium2 Bass kernel for a pre-LN transformer block (MHA + 3-layer FFN).

Sharding: data-parallel over (batch, query-chunk-pair) -> 8 cores, each
owning 1024 query tokens of one batch row.  To balance causal work, each
core owns a folded pair of 512-token chunks (chunk c and chunk 3-c of the
row), so both cores of a row see the same causal block structure.  K/V are
computed over the full 2048-token context per row (duplicated across the 2
cores sharing a row) - no cross-core collectives.  Context columns are
ordered own-chunks-first; the host-built per-core causal mask carries the
global positions.

On-device layout: all activations are TRANSPOSED, [feature, token],
features tiled 128-per-partition.  Every matmul is out^T = lhsT.T @ rhs
with lhsT = W in natural [in, out] layout, so the convention is maintained
end-to-end with zero on-chip transposes.  Attention scores are computed
transposed ([key, query]); softmax normalization reduces over the
partition (key) axis via a ones-column appended to V, so the denominator
falls out of the same PE accumulation that computes attn @ V.  LayerNorm
reduces over the partition (feature) axis with ones-vector matmuls
(float32r: single-pass fp32) on the PE.

Causal structure (uniform across cores by construction): s-tiles 0-7 are
the own chunks, 8-15 the other two chunks ascending.  Tiles {0-3, 8-11}
are needed by all 1024 queries ("full" units); tiles {4-7, 12-15} only by
the upper 512 queries ("half" units) - 25% of score/softmax/AV work is
statically skipped.
"""

import os

import numpy as np
import ml_dtypes

B, T, C = 4, 2048, 1024
H, D = 16, 64
FF = 4 * C
EPS = 1e-5
P = 128
CC = C // P          # 8 feature chunks
FOC = FF // P        # 32 ff chunks
TQ = 1024            # own (query) tokens per core
TC = 2048            # context tokens per core
NCORES = 8
SCALE = float(C) ** -0.5

FULL_ST = [0, 1, 2, 3, 8, 9, 10, 11]     # key tiles needed by all queries
HALF_ST = [4, 5, 6, 7, 12, 13, 14, 15]   # key tiles needed by queries 512:1024

BF16 = ml_dtypes.bfloat16

_PROG = None         # compiled Bacc program, built once per process
LAST_RESULT = None   # BassKernelResults of the most recent run (for profiling)


def _build_program():
    import concourse.mybir as mybir
    import concourse.tile as tile
    from concourse import bacc

    f32 = mybir.dt.float32
    f16 = mybir.dt.float16
    bf16 = mybir.dt.bfloat16
    AF = mybir.ActivationFunctionType
    ALU = mybir.AluOpType

    nc = bacc.Bacc("TRN2", target_bir_lowering=False, debug=False)

    # ---- DRAM parameters (per-core shapes; all pre-packed on host) ----
    xq = nc.declare_dram_parameter("xq", [P, CC, TQ], f32, isOutput=False)
    xTb = nc.declare_dram_parameter("xTb", [P, CC, TC], bf16, isOutput=False)
    maskT = nc.declare_dram_parameter("maskT", [P, 16, 512], bf16,
                                      isOutput=False)
    wq = nc.declare_dram_parameter("wq", [8, P, CC, P], bf16, isOutput=False)
    wk = nc.declare_dram_parameter("wk", [8, P, CC, P], bf16, isOutput=False)
    wv = nc.declare_dram_parameter("wv", [8, P, CC, 128], bf16, isOutput=False)
    wproj = nc.declare_dram_parameter("wproj", [P, CC, C], bf16, isOutput=False)
    w1 = nc.declare_dram_parameter("w1", [FOC, P, CC, P], bf16, isOutput=False)
    w2 = nc.declare_dram_parameter("w2", [FOC, P, FOC, P], bf16, isOutput=False)
    w3 = nc.declare_dram_parameter("w3", [CC, P, FOC, P], bf16, isOutput=False)
    bproj = nc.declare_dram_parameter("bproj", [P, CC], f32, isOutput=False)
    b1 = nc.declare_dram_parameter("b1", [P, FOC], f32, isOutput=False)
    b2 = nc.declare_dram_parameter("b2", [P, FOC], f32, isOutput=False)
    b3 = nc.declare_dram_parameter("b3", [P, CC], f32, isOutput=False)
    qb = nc.declare_dram_parameter("qb", [P, 8], f32, isOutput=False)
    kb = nc.declare_dram_parameter("kb", [P, 8], f32, isOutput=False)
    outT = nc.declare_dram_parameter("outT", [P, CC, TQ], f32, isOutput=True)

    TSW = 256  # layernorm column-slice width

    def layernorm_T(lp, lpr, lps, fill_xslice, ncols, out,
                    oc_col, ones_row, eps_sb):
        """Feature-axis LN (affine folded into downstream weights/biases on
        the host) of transposed bf16 activations, software-pipelined two
        slices deep: the normalize (PE broadcast + DVE) of slice ts-2 is
        emitted after the stats matmuls of slice ts so the in-order PE
        queue never stalls on the row-stats chain.  x and x^2 are packed
        adjacently per chunk so one bf16 ones-matmul (with 1/C folded into
        the ones value) yields both E[x] and E[x^2].  The reciprocal runs
        on the 128-lane broadcast of std, not the 1-lane row."""
        pends = []

        def emit_norm(p):
            ts, xsqs, mrow, std, sl = p
            ps_mu = lps.tile([P, TSW], f32, tag="ps_mu")
            nc.tensor.matmul(ps_mu[:], ones_row[:], mrow[:, 0:TSW],
                             start=True, stop=True)
            ps_rs = lps.tile([P, TSW], f32, tag="ps_rs")
            nc.tensor.matmul(ps_rs[:], ones_row[:], std[:],
                             start=True, stop=True)
            rsB = lp.tile([P, TSW], f32, tag="ln_rsB")
            nc.vector.reciprocal(rsB[:], ps_rs[:])
            for cc in range(CC):
                t1 = lp.tile([P, TSW], f32, tag="ln_t1")
                nc.vector.tensor_sub(t1[:], xsqs[cc][:, 0, :], ps_mu[:])
                nc.vector.tensor_mul(out[:, cc, sl], t1[:], rsB[:])

        for ts in range(ncols // TSW):
            sl = slice(ts * TSW, (ts + 1) * TSW)
            xsqs = [lp.tile([P, 2, TSW], bf16, tag=f"xsq{cc}", name=f"xsq{cc}")
                    for cc in range(CC)]
            fill_xslice(xsqs, ts, sl)
            ps_st = lps.tile([1, 2 * TSW], f32, tag="ps_st")
            for cc in range(CC):
                if cc < 4:
                    nc.scalar.square(xsqs[cc][:, 1, :], xsqs[cc][:, 0, :])
                else:
                    nc.vector.tensor_mul(xsqs[cc][:, 1, :], xsqs[cc][:, 0, :],
                                         xsqs[cc][:, 0, :])
                nc.tensor.matmul(ps_st[:], oc_col[:],
                                 xsqs[cc].rearrange("p a b -> p (a b)"),
                                 start=(cc == 0), stop=(cc == CC - 1))
            mrow = lpr.tile([1, 2 * TSW], f32, tag="ln_mrow")
            nc.vector.tensor_copy(mrow[:], ps_st[:])
            msq = lpr.tile([1, TSW], f32, tag="ln_msq")
            nc.vector.tensor_mul(msq[:], mrow[:, 0:TSW], mrow[:, 0:TSW])
            nc.vector.tensor_sub(mrow[:, TSW:2 * TSW],
                                 mrow[:, TSW:2 * TSW], msq[:])
            std = lpr.tile([1, TSW], f32, tag="ln_std")
            nc.scalar.activation(std[:], mrow[:, TSW:2 * TSW], AF.Sqrt,
                                 bias=eps_sb[:])
            pends.append((ts, xsqs, mrow, std, sl))
            if len(pends) > 2:
                emit_norm(pends.pop(0))
        for p in pends:
            emit_norm(p)

    with tile.TileContext(nc) as tc:
        with tc.tile_pool(name="const", bufs=1) as cp:
            oc_col = cp.tile([P, 1], bf16)
            nc.vector.memset(oc_col[:], 1.0 / C)
            ones_row = cp.tile([1, P], f32)
            nc.vector.memset(ones_row[:], 1.0)
            eps_sb = cp.tile([1, 1], f32)
            nc.vector.memset(eps_sb[:], EPS)
            ones_row_h = cp.tile([1, 64], f16)
            nc.vector.memset(ones_row_h[:], 1.0)

            qb_sb = cp.tile([P, 8], f32, tag="qb")
            nc.sync.dma_start(qb_sb[:], qb[:])
            kb_sb = cp.tile([P, 8], f32, tag="kb")
            nc.sync.dma_start(kb_sb[:], kb[:])
            bproj_sb = cp.tile([P, CC], f32, tag="bproj")
            nc.sync.dma_start(bproj_sb[:], bproj[:])
            b1_sb = cp.tile([P, FOC], f32, tag="b1")
            nc.sync.dma_start(b1_sb[:], b1[:])
            b2_sb = cp.tile([P, FOC], f32, tag="b2")
            nc.sync.dma_start(b2_sb[:], b2[:])
            b3_sb = cp.tile([P, CC], f32, tag="b3")
            nc.sync.dma_start(b3_sb[:], b3[:])

            with tc.tile_pool(name="x1", bufs=1) as p_x1:
                x1T = p_x1.tile([P, CC, TQ], f32, tag="x1T")

                # ======== attention residual branch ========
                with tc.tile_pool(name="ao", bufs=1) as p_ao:
                    attnT = p_ao.tile([P, CC, TQ], bf16, tag="attnT")

                    with tc.tile_pool(name="qkv", bufs=1) as qp:
                        kT = qp.tile([P, 8, TC], bf16, tag="kT")   # head pairs
                        qT = qp.tile([P, 8, TQ], bf16, tag="qT")
                        V_aug = qp.tile([P, 16, H, 65], bf16, tag="V_aug")
                        nc.vector.memset(V_aug[:], 1.0)  # ones col @ index 64

                        with tc.tile_pool(name="h", bufs=1) as php:
                            hT = php.tile([P, CC, TC], bf16, tag="hT")

                            # ---------------- LN1 over full context --------
                            with (
                                tc.tile_pool(name="ln1p", bufs=2) as lp,
                                tc.tile_pool(name="ln1r", bufs=2) as lpr,
                                tc.tile_pool(name="ln1ps", bufs=2,
                                             space="PSUM") as lps,
                            ):
                                def fill_x1slice(xsqs, ts, sl):
                                    for cc in range(CC):
                                        nc.sync.dma_start(xsqs[cc][:, 0, :],
                                                          xTb[:, cc, sl])
                                layernorm_T(lp, lpr, lps, fill_x1slice, TC,
                                            hT, oc_col, ones_row, eps_sb)

                            # ------- merged QKV + attention -------
                            # QKV work is cut into small self-contained
                            # matmul chains.  Pairs 0-1 and the first half of
                            # V run up front; the rest is drip-fed between
                            # attention units so the in-order PE queue never
                            # drains (keeps the HAM clock warm).
                            half_pairs = [(HALF_ST[i], HALF_ST[i + 1])
                                          for i in range(0, 8, 2)]
                            with (
                                tc.tile_pool(name="qkvw", bufs=2) as qwp,
                                tc.tile_pool(name="qkvps", bufs=3,
                                             space="PSUM") as qps,
                                tc.tile_pool(name="attn", bufs=2) as atp,
                                tc.tile_pool(name="attnm", bufs=1) as amp,
                                tc.tile_pool(name="attnps", bufs=1,
                                             space="PSUM") as aps,
                                tc.tile_pool(name="attnps1", bufs=1,
                                             space="PSUM") as aps1,
                            ):
                                mask_sb = amp.tile([P, 16, 512], bf16,
                                                   tag="mask")
                                nc.sync.dma_start(mask_sb[:], maskT[:])

                                # Each chain is split (load, run): the weight
                                # DMA is issued two chains ahead of its
                                # matmuls so the PE never waits on a load.
                                def kq_chain(wsrc, bias_sb, dest_kT, pair, js):
                                    sl = slice(js * 512, (js + 1) * 512)

                                    def load():
                                        wt = qwp.tile([P, CC, P], bf16,
                                                      tag="wkq", name="wkq")
                                        nc.sync.dma_start(wt[:], wsrc[pair])

                                        def run():
                                            ps_f = qps.tile([P, 512], f32,
                                                            tag="ps_f",
                                                            name="ps_f")
                                            for cc in range(CC):
                                                nc.tensor.matmul(
                                                    ps_f[:], wt[:, cc, :],
                                                    hT[:, cc, sl],
                                                    start=(cc == 0),
                                                    stop=(cc == CC - 1))
                                            nc.vector.tensor_scalar_add(
                                                dest_kT[:, pair, sl], ps_f[:],
                                                bias_sb[:, pair:pair + 1])
                                        return run
                                    return load

                                def v_chain(vj, sc):
                                    def load():
                                        wt = qwp.tile([P, CC, 128], bf16,
                                                      tag="wv", name="wv")
                                        nc.sync.dma_start(wt[:], wv[vj])

                                        def run():
                                            ps_f = qps.tile([P, 512], f32,
                                                            tag="ps_f",
                                                            name="ps_f")
                                            for cc in range(CC):
                                                nc.tensor.matmul(
                                                    ps_f[:, 0:128],
                                                    hT[:, cc,
                                                       sc * P:(sc + 1) * P],
                                                    wt[:, cc, :],
                                                    start=(cc == 0),
                                                    stop=(cc == CC - 1))
                                            for hh in range(2):
                                                hd = vj * 2 + hh
                                                nc.vector.tensor_copy(
                                                    V_aug[:, sc, hd, 0:64],
                                                    ps_f[:,
                                                         hh * 64:(hh + 1) * 64])
                                        return run
                                    return load

                                def pair_chains(pair):
                                    return ([kq_chain(wk, kb_sb, kT, pair, js)
                                             for js in range(4)]
                                            + [kq_chain(wq, qb_sb, qT, pair, js)
                                               for js in range(2)])

                                loaded = []

                                def drip(n):
                                    for _ in range(n):
                                        if fill:
                                            loaded.append(fill.pop()())
                                        if len(loaded) > 2:
                                            loaded.pop(0)()

                                def drain():
                                    while loaded:
                                        loaded.pop(0)()

                                # up-front: pairs 0-1 and V half 0
                                fill = pair_chains(0) + pair_chains(1)
                                for vj in range(4):
                                    for sc in range(16):
                                        fill.append(v_chain(vj, sc))
                                fill = list(reversed(fill))
                                drip(len(fill))
                                drain()

                                # drip-fed fillers with readiness deadlines:
                                # (chains-remaining ceiling before head h)
                                fill = []
                                fill += pair_chains(2) + pair_chains(3)
                                fill += pair_chains(4)
                                for vj in range(4, 8):
                                    for sc in range(16):
                                        fill.append(v_chain(vj, sc))
                                fill += (pair_chains(5) + pair_chains(6)
                                         + pair_chains(7))
                                n_fill = len(fill)   # 100
                                deadline = {4: n_fill - 6, 6: n_fill - 12,
                                            8: 18, 10: 12, 12: 6, 14: 0}
                                fill = list(reversed(fill))  # pop() from end

                                def pump(target_remaining):
                                    # run chains until at most
                                    # target_remaining are not yet executed
                                    while len(fill) + len(loaded) > \
                                            target_remaining:
                                        if fill and len(loaded) <= 2:
                                            loaded.append(fill.pop()())
                                        elif loaded:
                                            loaded.pop(0)()

                                pend = None

                                def emit_norm(p):
                                    dn, avf, h = p
                                    pair, off = h // 2, (h % 2) * 64
                                    for js in range(2):
                                        sl = slice(js * 512, (js + 1) * 512)
                                        ps_bc = aps1.tile([64, 512], f32,
                                                          tag="ps_bc",
                                                          name="ps_bc")
                                        nc.tensor.matmul(ps_bc[:],
                                                         ones_row_h[:],
                                                         dn[:, sl],
                                                         start=True, stop=True)
                                        rcB = atp.tile([64, 512], f32,
                                                       tag="rcB", name="rcB")
                                        nc.vector.reciprocal_approx_fast(
                                            rcB[:], ps_bc[:])
                                        nc.vector.tensor_mul(
                                            attnT[off:off + 64, pair, sl],
                                            avf[:, sl], rcB[:])

                                for h in range(H):
                                    pair, off = h // 2, (h % 2) * 64
                                    if h in deadline:
                                        pump(deadline[h])
                                    ps_av = aps1.tile([65, TQ], f32,
                                                      tag="ps_av")
                                    # 8 full units: all 1024 queries vs one
                                    # key tile
                                    for ui, st in enumerate(FULL_ST):
                                        if ui == 2 and pend is not None:
                                            emit_norm(pend)
                                            pend = None
                                        if h < 8 or ui % 2 == 1:
                                            drip(1)
                                        ps_s = aps.tile([P, TQ], f32,
                                                        tag="ps_s")
                                        for js in range(2):
                                            sl = slice(js * 512, (js + 1) * 512)
                                            nc.tensor.matmul(
                                                ps_s[:, sl],
                                                kT[off:off + 64, pair,
                                                   st * P:(st + 1) * P],
                                                qT[off:off + 64, pair, sl],
                                                start=True, stop=True)
                                        ex = atp.tile([P, TQ], bf16, tag="ex")
                                        nc.scalar.activation(ex[:], ps_s[:],
                                                             AF.Exp,
                                                             scale=SCALE)
                                        # only queries 0:512 can be masked
                                        # here; the upper half is statically
                                        # all-ones
                                        nc.vector.tensor_mul(
                                            ex[:, 0:512], ex[:, 0:512],
                                            mask_sb[:, st, :])
                                        for js in range(2):
                                            sl = slice(js * 512, (js + 1) * 512)
                                            nc.tensor.matmul(
                                                ps_av[:, sl],
                                                V_aug[:, st, h, :],
                                                ex[:, sl],
                                                start=(ui == 0),
                                                stop=(ui == 7 and js == 0))
                                    # 4 half-unit pairs: queries 512:1024 vs
                                    # two key tiles, sharing one psum/exp
                                    for pi, (stA, stB) in enumerate(half_pairs):
                                        if h < 8 or pi % 2 == 0:
                                            drip(1)
                                        ps_s = aps.tile([P, TQ], f32,
                                                        tag="ps_s")
                                        for js, st in ((0, stA), (1, stB)):
                                            sl = slice(js * 512, (js + 1) * 512)
                                            nc.tensor.matmul(
                                                ps_s[:, sl],
                                                kT[off:off + 64, pair,
                                                   st * P:(st + 1) * P],
                                                qT[off:off + 64, pair,
                                                   512:1024],
                                                start=True, stop=True)
                                        ex = atp.tile([P, TQ], bf16, tag="ex")
                                        nc.scalar.activation(ex[:], ps_s[:],
                                                             AF.Exp,
                                                             scale=SCALE)
                                        for js, st in ((0, stA), (1, stB)):
                                            sl = slice(js * 512, (js + 1) * 512)
                                            nc.vector.tensor_mul(
                                                ex[:, sl], ex[:, sl],
                                                mask_sb[:, st, :])
                                            nc.tensor.matmul(
                                                ps_av[:, 512:1024],
                                                V_aug[:, st, h, :], ex[:, sl],
                                                start=False,
                                                stop=(pi == 3 and js == 1))
                                    dn = atp.tile([1, TQ], f16, tag="dn")
                                    nc.vector.tensor_copy(dn[:],
                                                          ps_av[64:65, :])
                                    avf = atp.tile([64, TQ], bf16, tag="avf")
                                    nc.vector.tensor_copy(avf[:],
                                                          ps_av[0:64, :])
                                    pend = (dn, avf, h)
                                pump(0)
                                emit_norm(pend)

                    # ---------------- out-projection + residual ------------
                    with (
                        tc.tile_pool(name="proj", bufs=3) as pp,
                        tc.tile_pool(name="projw", bufs=1) as pwp,
                        tc.tile_pool(name="projps", bufs=4, space="PSUM") as pps,
                    ):
                        wp_sb = pwp.tile([P, CC, C], bf16, tag="wproj")
                        nc.sync.dma_start(wp_sb[:], wproj[:])
                        xown = pwp.tile([P, CC, TQ], f32, tag="xown")
                        nc.sync.dma_start(xown[:], xq[:])
                        for co in range(CC):
                            for js in range(2):
                                sl = slice(js * 512, (js + 1) * 512)
                                ps_p = pps.tile([P, 512], f32, tag="ps_p")
                                for ci in range(CC):
                                    nc.tensor.matmul(
                                        ps_p[:],
                                        wp_sb[:, ci, co * P:(co + 1) * P],
                                        attnT[:, ci, sl],
                                        start=(ci == 0), stop=(ci == CC - 1))
                                t1 = pp.tile([P, 512], f32, tag="pj_t1")
                                nc.vector.tensor_scalar_add(
                                    t1[:], ps_p[:], bproj_sb[:, co:co + 1])
                                nc.vector.tensor_add(x1T[:, co, sl], t1[:],
                                                     xown[:, co, sl])

                # ======== FFN residual branch ========
                with tc.tile_pool(name="h2", bufs=1) as p_h2:
                    h2T = p_h2.tile([P, FOC, TQ], bf16, tag="h2T")

                    with tc.tile_pool(name="f1", bufs=1) as fp_in:
                        h2in = fp_in.tile([P, CC, TQ], bf16, tag="h2in")
                        # -------- LN2 --------
                        with (
                            tc.tile_pool(name="ln2p", bufs=2) as lp2,
                            tc.tile_pool(name="ln2r", bufs=2) as lpr2,
                            tc.tile_pool(name="ln2ps", bufs=2,
                                         space="PSUM") as lps2,
                        ):
                            def fill_x2slice(xsqs, ts, sl):
                                for cc in range(CC):
                                    nc.vector.tensor_copy(xsqs[cc][:, 0, :],
                                                          x1T[:, cc, sl])
                            layernorm_T(lp2, lpr2, lps2, fill_x2slice, TQ,
                                        h2in, oc_col, ones_row, eps_sb)
                        # -------- Y1 = relu(h2in @ W1 + b1) --------
                        with (
                            tc.tile_pool(name="f1w", bufs=2) as fwp1,
                            tc.tile_pool(name="f1ps", bufs=4,
                                         space="PSUM") as fps1,
                        ):
                            for fo in range(FOC):
                                w1t = fwp1.tile([P, CC, P], bf16, tag="w1t")
                                nc.sync.dma_start(w1t[:], w1[fo])
                                for js in range(2):
                                    sl = slice(js * 512, (js + 1) * 512)
                                    ps1 = fps1.tile([P, 512], f32, tag="ps1")
                                    for cc in range(CC):
                                        nc.tensor.matmul(
                                            ps1[:], w1t[:, cc, :],
                                            h2in[:, cc, sl],
                                            start=(cc == 0), stop=(cc == CC - 1))
                                    nc.scalar.activation(
                                        h2T[:, fo, sl], ps1[:], AF.Relu,
                                        bias=b1_sb[:, fo:fo + 1])

                    # -------- Y2 = relu(h2T @ W2 + b2) --------
                    with tc.tile_pool(name="h3", bufs=1) as p_h3:
                        h3T = p_h3.tile([P, FOC, TQ], bf16, tag="h3T")
                        with (
                            tc.tile_pool(name="f2w", bufs=3) as fwp2,
                            tc.tile_pool(name="f2ps", bufs=4,
                                         space="PSUM") as fps2,
                        ):
                            for fo in range(FOC):
                                w2t = fwp2.tile([P, FOC, P], bf16, tag="w2t")
                                nc.sync.dma_start(w2t[:], w2[fo])
                                for js in range(2):
                                    sl = slice(js * 512, (js + 1) * 512)
                                    ps2 = fps2.tile([P, 512], f32, tag="ps2")
                                    for fi in range(FOC):
                                        nc.tensor.matmul(
                                            ps2[:], w2t[:, fi, :],
                                            h2T[:, fi, sl],
                                            start=(fi == 0),
                                            stop=(fi == FOC - 1))
                                    nc.scalar.activation(
                                        h3T[:, fo, sl], ps2[:], AF.Relu,
                                        bias=b2_sb[:, fo:fo + 1])

                        # -------- Y3 + bias + residual -> out --------
                        with (
                            tc.tile_pool(name="f3", bufs=3) as fp3,
                            tc.tile_pool(name="f3w", bufs=2) as fwp3,
                            tc.tile_pool(name="f3ps", bufs=4,
                                         space="PSUM") as fps3,
                        ):
                            for co in range(CC):
                                w3t = fwp3.tile([P, FOC, P], bf16, tag="w3t")
                                nc.sync.dma_start(w3t[:], w3[co])
                                for js in range(2):
                                    sl = slice(js * 512, (js + 1) * 512)
                                    ps3 = fps3.tile([P, 512], f32, tag="ps3")
                                    for fi in range(FOC):
                                        nc.tensor.matmul(
                                            ps3[:], w3t[:, fi, :],
                                            h3T[:, fi, sl],
                                            start=(fi == 0),
                                            stop=(fi == FOC - 1))
                                    ot = fp3.tile([P, 512], f32, tag="ot")
                                    nc.vector.tensor_scalar_add(
                                        ot[:], ps3[:], b3_sb[:, co:co + 1])
                                    nc.vector.tensor_add(ot[:], ot[:],
                                                         x1T[:, co, sl])
                                    nc.sync.dma_start(outT[:, co, sl], ot[:])

    nc.compile()
    return nc


def _pack_vec(v, nchunks):
    # [nchunks*P] -> [P, nchunks]
    return np.ascontiguousarray(np.asarray(v, dtype=np.float32).reshape(nchunks, P).T)


def _pack_w(w, in_chunks, out_dim):
    # [in, out] -> [P, in_chunks, out]  (lhsT tiles, fully resident)
    w = np.asarray(w).astype(BF16)
    return np.ascontiguousarray(w.reshape(in_chunks, P, out_dim).transpose(1, 0, 2))


def _pack_w_stream(w, in_chunks, out_chunks, out_w=P):
    # [in, out] -> [out_chunks, P, in_chunks, out_w]  (streamed lhsT tiles)
    w = np.asarray(w).astype(BF16)
    return np.ascontiguousarray(
        w.reshape(in_chunks, P, out_chunks, out_w).transpose(2, 1, 0, 3))


def _core_chunks(core):
    """Global 512-token chunk ids (of the batch row) in this core's context
    column order: own folded pair first, then the other two ascending."""
    c = core % 2
    own = [c, 3 - c]
    others = sorted(set(range(4)) - set(own))
    return own, others


def kernel(x, Wq, Wk, Wv, Wproj, bproj, W1, b1, W2, b2, W3, b3,
           ln1_g, ln1_b, ln2_g, ln2_b):
    global _PROG, LAST_RESULT
    from concourse.bass_utils import run_bass_kernel_spmd

    if _PROG is None:
        _PROG = _build_program()

    x = np.asarray(x, dtype=np.float32)

    # Fold the LN affine transforms into the consuming weights/biases:
    #   h_full = g * h_raw + b  with  h_raw = (x - mu) * rstd
    # Q/K/V/W1 rows get scaled by g; the b contribution becomes an additive
    # bias: per-d for Q/K (applied at PSUM eviction), constant-through-
    # softmax for V (folded into bproj via Wproj), and per-ff for W1.
    g1 = np.asarray(ln1_g, dtype=np.float32)[:, None]
    b1v = np.asarray(ln1_b, dtype=np.float32)
    g2 = np.asarray(ln2_g, dtype=np.float32)[:, None]
    b2v = np.asarray(ln2_b, dtype=np.float32)
    Wq2 = np.asarray(Wq, dtype=np.float32).transpose(1, 0, 2).reshape(C, C)
    Wk2 = np.asarray(Wk, dtype=np.float32).transpose(1, 0, 2).reshape(C, C)
    Wv2 = np.asarray(Wv, dtype=np.float32).transpose(1, 0, 2).reshape(C, C)
    W1f = np.asarray(W1, dtype=np.float32)
    vb = b1v @ Wv2                                   # [C], per-(h,d) V bias
    bproj_f = np.asarray(bproj, dtype=np.float32) + vb @ np.asarray(
        Wproj, dtype=np.float32)
    b1_f = np.asarray(b1, dtype=np.float32) + b2v @ W1f

    common = {
        "wq": _pack_w_stream(g1 * Wq2, CC, 8),
        "wk": _pack_w_stream(g1 * Wk2, CC, 8),
        "wv": _pack_w_stream(g1 * Wv2, CC, 8, 128),
        "wproj": _pack_w(Wproj, CC, C),
        "w1": _pack_w_stream(g2 * W1f, CC, FOC),
        "w2": _pack_w_stream(W2, FOC, FOC),
        "w3": _pack_w_stream(W3, FOC, CC),
        "qb": _pack_vec(b1v @ Wq2, 8),
        "kb": _pack_vec(b1v @ Wk2, 8),
        "bproj": _pack_vec(bproj_f, CC),
        "b1": _pack_vec(b1_f, FOC), "b2": _pack_vec(b2, FOC),
        "b3": _pack_vec(b3, CC),
    }

    ar512 = np.arange(512)
    in_maps = []
    for core in range(NCORES):
        b = core // 2
        own, others = _core_chunks(core)
        order = own + others
        xcat = np.concatenate([x[b, ch * 512:(ch + 1) * 512] for ch in order],
                              axis=0)                       # [2048, C]
        xT_p = np.ascontiguousarray(
            xcat.T.reshape(CC, P, TC).transpose(1, 0, 2))   # [P, CC, TC]
        kpos = np.concatenate([ch * 512 + ar512 for ch in order])
        qpos = np.concatenate([ch * 512 + ar512 for ch in own])
        mask = (kpos[:, None] <= qpos[None, :]).astype(BF16)      # [TC, TQ]
        mask16 = mask.reshape(16, P, TQ)
        # each key tile only ever sees one query half: lower (0:512) for
        # "full" units, upper (512:1024) for "half" units
        mask_p = np.empty((P, 16, 512), dtype=BF16)
        for st in range(16):
            q0 = 0 if st in FULL_ST else 512
            mask_p[:, st, :] = mask16[st, :, q0:q0 + 512]
        in_maps.append({**common,
                        "xq": np.ascontiguousarray(xT_p[:, :, 0:TQ]),
                        "xTb": xT_p.astype(BF16),
                        "maskT": mask_p})

    trace = bool(os.environ.get("BASS_TRACE"))
    res = run_bass_kernel_spmd(_PROG, in_maps, core_ids=list(range(NCORES)),
                               trace=trace)
    LAST_RESULT = res

    out = np.empty((B, T, C), dtype=np.float32)
    for core in range(NCORES):
        b = core // 2
        own, _ = _core_chunks(core)
        oT = res.results[core]["outT"]                      # [P, CC, TQ]
        o2 = oT.transpose(2, 1, 0).reshape(TQ, C)           # [q, C]
        for i, ch in enumerate(own):
            out[b, ch * 512:(ch + 1) * 512] = o2[i * 512:(i + 1) * 512]
    return out


# revision 67
# speedup vs baseline: 1.0901x; 1.0837x over previous
"""Trainium2 Bass kernel for a pre-LN transformer block (MHA + 3-layer FFN).

Sharding: data-parallel over (batch, query-chunk-pair) -> 8 cores, each
owning 1024 query tokens of one batch row.  To balance causal work, each
core owns a folded pair of 512-token chunks (chunk c and chunk 3-c of the
row), so both cores of a row see the same causal block structure.  K/V are
computed over the full 2048-token context per row (duplicated across the 2
cores sharing a row) - no cross-core collectives.  Context columns are
ordered own-chunks-first; the host-built per-core causal mask carries the
global positions.

On-device layout: all activations are TRANSPOSED, [feature, token],
features tiled 128-per-partition.  Every matmul is out^T = lhsT.T @ rhs
with lhsT = W in natural [in, out] layout, so the convention is maintained
end-to-end with zero on-chip transposes.  Attention scores are computed
transposed ([key, query]); softmax normalization reduces over the
partition (key) axis via a ones-column appended to V, so the denominator
falls out of the same PE accumulation that computes attn @ V.  LayerNorm
reduces over the partition (feature) axis with ones-vector matmuls
(float32r: single-pass fp32) on the PE.

Causal structure (uniform across cores by construction): s-tiles 0-7 are
the own chunks, 8-15 the other two chunks ascending.  Tiles {0-3, 8-11}
are needed by all 1024 queries ("full" units); tiles {4-7, 12-15} only by
the upper 512 queries ("half" units) - 25% of score/softmax/AV work is
statically skipped.
"""

import os

import numpy as np
import ml_dtypes

B, T, C = 4, 2048, 1024
H, D = 16, 64
FF = 4 * C
EPS = 1e-5
P = 128
CC = C // P          # 8 feature chunks
FOC = FF // P        # 32 ff chunks
TQ = 1024            # own (query) tokens per core
TC = 2048            # context tokens per core
NCORES = 8
SCALE = float(C) ** -0.5

FULL_ST = [0, 1, 2, 3, 8, 9, 10, 11]     # key tiles needed by all queries
HALF_ST = [4, 5, 6, 7, 12, 13, 14, 15]   # key tiles needed by queries 512:1024

BF16 = ml_dtypes.bfloat16

_PROG = None         # compiled Bacc program, built once per process
LAST_RESULT = None   # BassKernelResults of the most recent run (for profiling)


def _build_program():
    import concourse.mybir as mybir
    import concourse.tile as tile
    from concourse import bacc

    f32 = mybir.dt.float32
    f16 = mybir.dt.float16
    bf16 = mybir.dt.bfloat16
    AF = mybir.ActivationFunctionType
    ALU = mybir.AluOpType

    nc = bacc.Bacc("TRN2", target_bir_lowering=False, debug=False)

    # ---- DRAM parameters (per-core shapes; all pre-packed on host) ----
    xq = nc.declare_dram_parameter("xq", [P, CC, TQ], f32, isOutput=False)
    xTb = nc.declare_dram_parameter("xTb", [P, CC, TC], bf16, isOutput=False)
    maskT = nc.declare_dram_parameter("maskT", [P, 16, 512], bf16,
                                      isOutput=False)
    wq = nc.declare_dram_parameter("wq", [8, P, CC, P], bf16, isOutput=False)
    wk = nc.declare_dram_parameter("wk", [8, P, CC, P], bf16, isOutput=False)
    wv = nc.declare_dram_parameter("wv", [8, P, CC, 128], bf16, isOutput=False)
    wproj = nc.declare_dram_parameter("wproj", [P, CC, C], bf16, isOutput=False)
    w1 = nc.declare_dram_parameter("w1", [FOC, P, CC, P], bf16, isOutput=False)
    w2 = nc.declare_dram_parameter("w2", [FOC, P, FOC, P], bf16, isOutput=False)
    w3 = nc.declare_dram_parameter("w3", [CC, P, FOC, P], bf16, isOutput=False)
    bproj = nc.declare_dram_parameter("bproj", [P, CC], f32, isOutput=False)
    b1 = nc.declare_dram_parameter("b1", [P, FOC], f32, isOutput=False)
    b2 = nc.declare_dram_parameter("b2", [P, FOC], f32, isOutput=False)
    b3 = nc.declare_dram_parameter("b3", [P, CC], f32, isOutput=False)
    qb = nc.declare_dram_parameter("qb", [P, 8], f32, isOutput=False)
    kb = nc.declare_dram_parameter("kb", [P, 8], f32, isOutput=False)
    outT = nc.declare_dram_parameter("outT", [P, CC, TQ], f32, isOutput=True)

    TSW = 256  # layernorm column-slice width

    def layernorm_T(lp, lpr, lps, fill_xslice, ncols, out,
                    oc_col, ones_row, eps_sb):
        """Feature-axis LN (affine folded into downstream weights/biases on
        the host) of transposed bf16 activations, software-pipelined two
        slices deep: the normalize (PE broadcast + DVE) of slice ts-2 is
        emitted after the stats matmuls of slice ts so the in-order PE
        queue never stalls on the row-stats chain.  x and x^2 are packed
        adjacently per chunk so one bf16 ones-matmul (with 1/C folded into
        the ones value) yields both E[x] and E[x^2].  The reciprocal runs
        on the 128-lane broadcast of std, not the 1-lane row."""
        pends = []

        def emit_norm(p):
            ts, xsqs, mrow, std, sl = p
            ps_mu = lps.tile([P, TSW], f32, tag="ps_mu")
            nc.tensor.matmul(ps_mu[:], ones_row[:], mrow[:, 0:TSW],
                             start=True, stop=True)
            ps_rs = lps.tile([P, TSW], f32, tag="ps_rs")
            nc.tensor.matmul(ps_rs[:], ones_row[:], std[:],
                             start=True, stop=True)
            rsB = lp.tile([P, TSW], f32, tag="ln_rsB")
            nc.vector.reciprocal(rsB[:], ps_rs[:])
            for cc in range(CC):
                t1 = lp.tile([P, TSW], f32, tag="ln_t1")
                nc.vector.tensor_sub(t1[:], xsqs[cc][:, 0, :], ps_mu[:])
                nc.vector.tensor_mul(out[:, cc, sl], t1[:], rsB[:])

        for ts in range(ncols // TSW):
            sl = slice(ts * TSW, (ts + 1) * TSW)
            xsqs = [lp.tile([P, 2, TSW], bf16, tag=f"xsq{cc}", name=f"xsq{cc}")
                    for cc in range(CC)]
            fill_xslice(xsqs, ts, sl)
            ps_st = lps.tile([1, 2 * TSW], f32, tag="ps_st")
            for cc in range(CC):
                if cc < 4:
                    nc.scalar.square(xsqs[cc][:, 1, :], xsqs[cc][:, 0, :])
                else:
                    nc.vector.tensor_mul(xsqs[cc][:, 1, :], xsqs[cc][:, 0, :],
                                         xsqs[cc][:, 0, :])
                nc.tensor.matmul(ps_st[:], oc_col[:],
                                 xsqs[cc].rearrange("p a b -> p (a b)"),
                                 start=(cc == 0), stop=(cc == CC - 1))
            mrow = lpr.tile([1, 2 * TSW], f32, tag="ln_mrow")
            nc.vector.tensor_copy(mrow[:], ps_st[:])
            msq = lpr.tile([1, TSW], f32, tag="ln_msq")
            nc.vector.tensor_mul(msq[:], mrow[:, 0:TSW], mrow[:, 0:TSW])
            nc.vector.tensor_sub(mrow[:, TSW:2 * TSW],
                                 mrow[:, TSW:2 * TSW], msq[:])
            std = lpr.tile([1, TSW], f32, tag="ln_std")
            nc.scalar.activation(std[:], mrow[:, TSW:2 * TSW], AF.Sqrt,
                                 bias=eps_sb[:])
            pends.append((ts, xsqs, mrow, std, sl))
            if len(pends) > 2:
                emit_norm(pends.pop(0))
        for p in pends:
            emit_norm(p)

    with tile.TileContext(nc) as tc:
        with tc.tile_pool(name="const", bufs=1) as cp:
            oc_col = cp.tile([P, 1], bf16)
            nc.vector.memset(oc_col[:], 1.0 / C)
            ones_row = cp.tile([1, P], f32)
            nc.vector.memset(ones_row[:], 1.0)
            eps_sb = cp.tile([1, 1], f32)
            nc.vector.memset(eps_sb[:], EPS)
            ones_row_h = cp.tile([1, 64], f16)
            nc.vector.memset(ones_row_h[:], 1.0)

            qb_sb = cp.tile([P, 8], f32, tag="qb")
            nc.sync.dma_start(qb_sb[:], qb[:])
            kb_sb = cp.tile([P, 8], f32, tag="kb")
            nc.sync.dma_start(kb_sb[:], kb[:])
            bproj_sb = cp.tile([P, CC], f32, tag="bproj")
            nc.sync.dma_start(bproj_sb[:], bproj[:])
            b1_sb = cp.tile([P, FOC], f32, tag="b1")
            nc.sync.dma_start(b1_sb[:], b1[:])
            b2_sb = cp.tile([P, FOC], f32, tag="b2")
            nc.sync.dma_start(b2_sb[:], b2[:])
            b3_sb = cp.tile([P, CC], f32, tag="b3")
            nc.sync.dma_start(b3_sb[:], b3[:])

            with tc.tile_pool(name="x1", bufs=1) as p_x1:
                x1T = p_x1.tile([P, CC, TQ], f32, tag="x1T")

                # ======== attention residual branch ========
                with tc.tile_pool(name="ao", bufs=1) as p_ao:
                    attnT = p_ao.tile([P, CC, TQ], bf16, tag="attnT")

                    with tc.tile_pool(name="qkv", bufs=1) as qp:
                        kT = qp.tile([P, 8, TC], bf16, tag="kT")   # head pairs
                        qT = qp.tile([P, 8, TQ], bf16, tag="qT")
                        V_aug = qp.tile([P, 16, H, 65], bf16, tag="V_aug")
                        nc.vector.memset(V_aug[:], 1.0)  # ones col @ index 64

                        with tc.tile_pool(name="h", bufs=1) as php:
                            hT = php.tile([P, CC, TC], bf16, tag="hT")

                            # ---------------- LN1 over full context --------
                            with (
                                tc.tile_pool(name="ln1p", bufs=3) as lp,
                                tc.tile_pool(name="ln1r", bufs=3) as lpr,
                                tc.tile_pool(name="ln1ps", bufs=2,
                                             space="PSUM") as lps,
                            ):
                                def fill_x1slice(xsqs, ts, sl):
                                    for cc in range(CC):
                                        nc.sync.dma_start(xsqs[cc][:, 0, :],
                                                          xTb[:, cc, sl])
                                layernorm_T(lp, lpr, lps, fill_x1slice, TC,
                                            hT, oc_col, ones_row, eps_sb)

                            # ------- merged QKV + attention -------
                            # QKV work is cut into small self-contained
                            # matmul chains.  Pairs 0-1 and the first half of
                            # V run up front; the rest is drip-fed between
                            # attention units so the in-order PE queue never
                            # drains (keeps the HAM clock warm).
                            half_pairs = [(HALF_ST[i], HALF_ST[i + 1])
                                          for i in range(0, 8, 2)]
                            with (
                                tc.tile_pool(name="qkvw", bufs=2) as qwp,
                                tc.tile_pool(name="qkvps", bufs=2,
                                             space="PSUM") as qps,
                                tc.tile_pool(name="attn", bufs=2) as atp,
                                tc.tile_pool(name="attnm", bufs=1) as amp,
                                tc.tile_pool(name="attnps", bufs=2,
                                             space="PSUM") as aps,
                                tc.tile_pool(name="attnps1", bufs=1,
                                             space="PSUM") as aps1,
                            ):
                                mask_sb = amp.tile([P, 16, 512], bf16,
                                                   tag="mask")
                                nc.sync.dma_start(mask_sb[:], maskT[:])

                                # Each chain is split (load, run): the weight
                                # DMA is issued two chains ahead of its
                                # matmuls so the PE never waits on a load.
                                def kq_chain(wsrc, bias_sb, dest_kT, pair, js):
                                    sl = slice(js * 512, (js + 1) * 512)

                                    def load():
                                        wt = qwp.tile([P, CC, P], bf16,
                                                      tag="wkq", name="wkq")
                                        nc.sync.dma_start(wt[:], wsrc[pair])

                                        def run():
                                            ps_f = qps.tile([P, 512], f32,
                                                            tag="ps_f",
                                                            name="ps_f")
                                            for cc in range(CC):
                                                nc.tensor.matmul(
                                                    ps_f[:], wt[:, cc, :],
                                                    hT[:, cc, sl],
                                                    start=(cc == 0),
                                                    stop=(cc == CC - 1))
                                            nc.vector.tensor_scalar_add(
                                                dest_kT[:, pair, sl], ps_f[:],
                                                bias_sb[:, pair:pair + 1])
                                        return run
                                    return load

                                def v_chain(vj, sc):
                                    def load():
                                        wt = qwp.tile([P, CC, 128], bf16,
                                                      tag="wv", name="wv")
                                        nc.sync.dma_start(wt[:], wv[vj])

                                        def run():
                                            ps_f = qps.tile([P, 512], f32,
                                                            tag="ps_f",
                                                            name="ps_f")
                                            for cc in range(CC):
                                                nc.tensor.matmul(
                                                    ps_f[:, 0:128],
                                                    hT[:, cc,
                                                       sc * P:(sc + 1) * P],
                                                    wt[:, cc, :],
                                                    start=(cc == 0),
                                                    stop=(cc == CC - 1))
                                            for hh in range(2):
                                                hd = vj * 2 + hh
                                                nc.vector.tensor_copy(
                                                    V_aug[:, sc, hd, 0:64],
                                                    ps_f[:,
                                                         hh * 64:(hh + 1) * 64])
                                        return run
                                    return load

                                def pair_chains(pair):
                                    return ([kq_chain(wk, kb_sb, kT, pair, js)
                                             for js in range(4)]
                                            + [kq_chain(wq, qb_sb, qT, pair, js)
                                               for js in range(2)])

                                loaded = []

                                def drip(n):
                                    for _ in range(n):
                                        if fill:
                                            loaded.append(fill.pop()())
                                        if len(loaded) > 2:
                                            loaded.pop(0)()

                                def drain():
                                    while loaded:
                                        loaded.pop(0)()

                                # up-front: pairs 0-1 and V half 0
                                fill = pair_chains(0) + pair_chains(1)
                                for vj in range(4):
                                    for sc in range(16):
                                        fill.append(v_chain(vj, sc))
                                fill = list(reversed(fill))
                                drip(len(fill))
                                drain()

                                # drip-fed fillers with readiness deadlines:
                                # (chains-remaining ceiling before head h)
                                fill = []
                                fill += pair_chains(2) + pair_chains(3)
                                fill += pair_chains(4)
                                for vj in range(4, 8):
                                    for sc in range(16):
                                        fill.append(v_chain(vj, sc))
                                fill += (pair_chains(5) + pair_chains(6)
                                         + pair_chains(7))
                                n_fill = len(fill)   # 100
                                deadline = {4: n_fill - 6, 6: n_fill - 12,
                                            8: 18, 10: 12, 12: 6, 14: 0}
                                fill = list(reversed(fill))  # pop() from end

                                def pump(target_remaining):
                                    # run chains until at most
                                    # target_remaining are not yet executed
                                    while len(fill) + len(loaded) > \
                                            target_remaining:
                                        if fill and len(loaded) <= 2:
                                            loaded.append(fill.pop()())
                                        elif loaded:
                                            loaded.pop(0)()

                                pend = None

                                def emit_norm(p):
                                    dn, avf, h = p
                                    pair, off = h // 2, (h % 2) * 64
                                    for js in range(2):
                                        sl = slice(js * 512, (js + 1) * 512)
                                        ps_bc = qps.tile([P, 512], f32,
                                                         tag="ps_f",
                                                         name="ps_f")
                                        nc.tensor.matmul(ps_bc[0:64, :],
                                                         ones_row_h[:],
                                                         dn[:, sl],
                                                         start=True, stop=True)
                                        rcB = atp.tile([64, 512], f32,
                                                       tag="rcB", name="rcB")
                                        nc.vector.reciprocal_approx_fast(
                                            rcB[:], ps_bc[0:64, :])
                                        nc.vector.tensor_mul(
                                            attnT[off:off + 64, pair, sl],
                                            avf[:, sl], rcB[:])

                                for h in range(H):
                                    pair, off = h // 2, (h % 2) * 64
                                    if h in deadline:
                                        pump(deadline[h])
                                    ps_av = aps1.tile([65, TQ], f32,
                                                      tag="ps_av")
                                    # 8 full units: all 1024 queries vs one
                                    # key tile
                                    for ui, st in enumerate(FULL_ST):
                                        if ui == 2 and pend is not None:
                                            emit_norm(pend)
                                            pend = None
                                        if h < 8 or ui % 2 == 1:
                                            drip(1)
                                        ps_s = aps.tile([P, TQ], f32,
                                                        tag="ps_s")
                                        for js in range(2):
                                            sl = slice(js * 512, (js + 1) * 512)
                                            nc.tensor.matmul(
                                                ps_s[:, sl],
                                                kT[off:off + 64, pair,
                                                   st * P:(st + 1) * P],
                                                qT[off:off + 64, pair, sl],
                                                start=True, stop=True)
                                        ex = atp.tile([P, TQ], bf16, tag="ex")
                                        nc.scalar.activation(ex[:], ps_s[:],
                                                             AF.Exp,
                                                             scale=SCALE)
                                        # only queries 0:512 can be masked
                                        # here; the upper half is statically
                                        # all-ones
                                        nc.vector.tensor_mul(
                                            ex[:, 0:512], ex[:, 0:512],
                                            mask_sb[:, st, :])
                                        for js in range(2):
                                            sl = slice(js * 512, (js + 1) * 512)
                                            nc.tensor.matmul(
                                                ps_av[:, sl],
                                                V_aug[:, st, h, :],
                                                ex[:, sl],
                                                start=(ui == 0),
                                                stop=(ui == 7 and js == 0))
                                    # 4 half-unit pairs: queries 512:1024 vs
                                    # two key tiles, sharing one psum/exp
                                    for pi, (stA, stB) in enumerate(half_pairs):
                                        if h < 8 or pi % 2 == 0:
                                            drip(1)
                                        ps_s = aps.tile([P, TQ], f32,
                                                        tag="ps_s")
                                        for js, st in ((0, stA), (1, stB)):
                                            sl = slice(js * 512, (js + 1) * 512)
                                            nc.tensor.matmul(
                                                ps_s[:, sl],
                                                kT[off:off + 64, pair,
                                                   st * P:(st + 1) * P],
                                                qT[off:off + 64, pair,
                                                   512:1024],
                                                start=True, stop=True)
                                        ex = atp.tile([P, TQ], bf16, tag="ex")
                                        nc.scalar.activation(ex[:], ps_s[:],
                                                             AF.Exp,
                                                             scale=SCALE)
                                        for js, st in ((0, stA), (1, stB)):
                                            sl = slice(js * 512, (js + 1) * 512)
                                            nc.vector.tensor_mul(
                                                ex[:, sl], ex[:, sl],
                                                mask_sb[:, st, :])
                                            nc.tensor.matmul(
                                                ps_av[:, 512:1024],
                                                V_aug[:, st, h, :], ex[:, sl],
                                                start=False,
                                                stop=(pi == 3 and js == 1))
                                    dn = atp.tile([1, TQ], f16, tag="dn")
                                    nc.vector.tensor_copy(dn[:],
                                                          ps_av[64:65, :])
                                    avf = atp.tile([64, TQ], bf16, tag="avf")
                                    nc.vector.tensor_copy(avf[:],
                                                          ps_av[0:64, :])
                                    pend = (dn, avf, h)
                                pump(0)
                                emit_norm(pend)

                    # ---------------- out-projection + residual ------------
                    with (
                        tc.tile_pool(name="proj", bufs=3) as pp,
                        tc.tile_pool(name="projw", bufs=1) as pwp,
                        tc.tile_pool(name="projps", bufs=4, space="PSUM") as pps,
                    ):
                        wp_sb = pwp.tile([P, CC, C], bf16, tag="wproj")
                        nc.sync.dma_start(wp_sb[:], wproj[:])
                        xown = pwp.tile([P, CC, TQ], f32, tag="xown")
                        nc.sync.dma_start(xown[:], xq[:])
                        for co in range(CC):
                            for js in range(2):
                                sl = slice(js * 512, (js + 1) * 512)
                                ps_p = pps.tile([P, 512], f32, tag="ps_p")
                                for ci in range(CC):
                                    nc.tensor.matmul(
                                        ps_p[:],
                                        wp_sb[:, ci, co * P:(co + 1) * P],
                                        attnT[:, ci, sl],
                                        start=(ci == 0), stop=(ci == CC - 1))
                                t1 = pp.tile([P, 512], f32, tag="pj_t1")
                                nc.vector.tensor_scalar_add(
                                    t1[:], ps_p[:], bproj_sb[:, co:co + 1])
                                nc.vector.tensor_add(x1T[:, co, sl], t1[:],
                                                     xown[:, co, sl])

                # ======== FFN residual branch ========
                with tc.tile_pool(name="h2", bufs=1) as p_h2:
                    h2T = p_h2.tile([P, FOC, TQ], bf16, tag="h2T")

                    with tc.tile_pool(name="f1", bufs=1) as fp_in:
                        h2in = fp_in.tile([P, CC, TQ], bf16, tag="h2in")
                        # -------- LN2 --------
                        with (
                            tc.tile_pool(name="ln2p", bufs=3) as lp2,
                            tc.tile_pool(name="ln2r", bufs=3) as lpr2,
                            tc.tile_pool(name="ln2ps", bufs=2,
                                         space="PSUM") as lps2,
                        ):
                            def fill_x2slice(xsqs, ts, sl):
                                for cc in range(CC):
                                    nc.vector.tensor_copy(xsqs[cc][:, 0, :],
                                                          x1T[:, cc, sl])
                            layernorm_T(lp2, lpr2, lps2, fill_x2slice, TQ,
                                        h2in, oc_col, ones_row, eps_sb)
                        # -------- Y1 = relu(h2in @ W1 + b1) --------
                        with (
                            tc.tile_pool(name="f1w", bufs=2) as fwp1,
                            tc.tile_pool(name="f1ps", bufs=4,
                                         space="PSUM") as fps1,
                        ):
                            for fo in range(FOC):
                                w1t = fwp1.tile([P, CC, P], bf16, tag="w1t")
                                nc.sync.dma_start(w1t[:], w1[fo])
                                for js in range(2):
                                    sl = slice(js * 512, (js + 1) * 512)
                                    ps1 = fps1.tile([P, 512], f32, tag="ps1")
                                    for cc in range(CC):
                                        nc.tensor.matmul(
                                            ps1[:], w1t[:, cc, :],
                                            h2in[:, cc, sl],
                                            start=(cc == 0), stop=(cc == CC - 1))
                                    nc.scalar.activation(
                                        h2T[:, fo, sl], ps1[:], AF.Relu,
                                        bias=b1_sb[:, fo:fo + 1])

                    # -------- Y2 = relu(h2T @ W2 + b2) --------
                    with tc.tile_pool(name="h3", bufs=1) as p_h3:
                        h3T = p_h3.tile([P, FOC, TQ], bf16, tag="h3T")
                        with (
                            tc.tile_pool(name="f2w", bufs=3) as fwp2,
                            tc.tile_pool(name="f2ps", bufs=4,
                                         space="PSUM") as fps2,
                        ):
                            for fo in range(FOC):
                                w2t = fwp2.tile([P, FOC, P], bf16, tag="w2t")
                                nc.sync.dma_start(w2t[:], w2[fo])
                                for js in range(2):
                                    sl = slice(js * 512, (js + 1) * 512)
                                    ps2 = fps2.tile([P, 512], f32, tag="ps2")
                                    for fi in range(FOC):
                                        nc.tensor.matmul(
                                            ps2[:], w2t[:, fi, :],
                                            h2T[:, fi, sl],
                                            start=(fi == 0),
                                            stop=(fi == FOC - 1))
                                    nc.scalar.activation(
                                        h3T[:, fo, sl], ps2[:], AF.Relu,
                                        bias=b2_sb[:, fo:fo + 1])

                        # -------- Y3 + bias + residual -> out --------
                        with (
                            tc.tile_pool(name="f3", bufs=3) as fp3,
                            tc.tile_pool(name="f3w", bufs=2) as fwp3,
                            tc.tile_pool(name="f3ps", bufs=4,
                                         space="PSUM") as fps3,
                        ):
                            for co in range(CC):
                                w3t = fwp3.tile([P, FOC, P], bf16, tag="w3t")
                                nc.sync.dma_start(w3t[:], w3[co])
                                for js in range(2):
                                    sl = slice(js * 512, (js + 1) * 512)
                                    ps3 = fps3.tile([P, 512], f32, tag="ps3")
                                    for fi in range(FOC):
                                        nc.tensor.matmul(
                                            ps3[:], w3t[:, fi, :],
                                            h3T[:, fi, sl],
                                            start=(fi == 0),
                                            stop=(fi == FOC - 1))
                                    ot = fp3.tile([P, 512], f32, tag="ot")
                                    nc.vector.tensor_scalar_add(
                                        ot[:], ps3[:], b3_sb[:, co:co + 1])
                                    nc.vector.tensor_add(ot[:], ot[:],
                                                         x1T[:, co, sl])
                                    nc.sync.dma_start(outT[:, co, sl], ot[:])

    nc.compile()
    return nc


def _pack_vec(v, nchunks):
    # [nchunks*P] -> [P, nchunks]
    return np.ascontiguousarray(np.asarray(v, dtype=np.float32).reshape(nchunks, P).T)


def _pack_w(w, in_chunks, out_dim):
    # [in, out] -> [P, in_chunks, out]  (lhsT tiles, fully resident)
    w = np.asarray(w).astype(BF16)
    return np.ascontiguousarray(w.reshape(in_chunks, P, out_dim).transpose(1, 0, 2))


def _pack_w_stream(w, in_chunks, out_chunks, out_w=P):
    # [in, out] -> [out_chunks, P, in_chunks, out_w]  (streamed lhsT tiles)
    w = np.asarray(w).astype(BF16)
    return np.ascontiguousarray(
        w.reshape(in_chunks, P, out_chunks, out_w).transpose(2, 1, 0, 3))


def _core_chunks(core):
    """Global 512-token chunk ids (of the batch row) in this core's context
    column order: own folded pair first, then the other two ascending."""
    c = core % 2
    own = [c, 3 - c]
    others = sorted(set(range(4)) - set(own))
    return own, others


def kernel(x, Wq, Wk, Wv, Wproj, bproj, W1, b1, W2, b2, W3, b3,
           ln1_g, ln1_b, ln2_g, ln2_b):
    global _PROG, LAST_RESULT
    from concourse.bass_utils import run_bass_kernel_spmd

    if _PROG is None:
        _PROG = _build_program()

    x = np.asarray(x, dtype=np.float32)

    # Fold the LN affine transforms into the consuming weights/biases:
    #   h_full = g * h_raw + b  with  h_raw = (x - mu) * rstd
    # Q/K/V/W1 rows get scaled by g; the b contribution becomes an additive
    # bias: per-d for Q/K (applied at PSUM eviction), constant-through-
    # softmax for V (folded into bproj via Wproj), and per-ff for W1.
    g1 = np.asarray(ln1_g, dtype=np.float32)[:, None]
    b1v = np.asarray(ln1_b, dtype=np.float32)
    g2 = np.asarray(ln2_g, dtype=np.float32)[:, None]
    b2v = np.asarray(ln2_b, dtype=np.float32)
    Wq2 = np.asarray(Wq, dtype=np.float32).transpose(1, 0, 2).reshape(C, C)
    Wk2 = np.asarray(Wk, dtype=np.float32).transpose(1, 0, 2).reshape(C, C)
    Wv2 = np.asarray(Wv, dtype=np.float32).transpose(1, 0, 2).reshape(C, C)
    W1f = np.asarray(W1, dtype=np.float32)
    vb = b1v @ Wv2                                   # [C], per-(h,d) V bias
    bproj_f = np.asarray(bproj, dtype=np.float32) + vb @ np.asarray(
        Wproj, dtype=np.float32)
    b1_f = np.asarray(b1, dtype=np.float32) + b2v @ W1f

    common = {
        "wq": _pack_w_stream(g1 * Wq2, CC, 8),
        "wk": _pack_w_stream(g1 * Wk2, CC, 8),
        "wv": _pack_w_stream(g1 * Wv2, CC, 8, 128),
        "wproj": _pack_w(Wproj, CC, C),
        "w1": _pack_w_stream(g2 * W1f, CC, FOC),
        "w2": _pack_w_stream(W2, FOC, FOC),
        "w3": _pack_w_stream(W3, FOC, CC),
        "qb": _pack_vec(b1v @ Wq2, 8),
        "kb": _pack_vec(b1v @ Wk2, 8),
        "bproj": _pack_vec(bproj_f, CC),
        "b1": _pack_vec(b1_f, FOC), "b2": _pack_vec(b2, FOC),
        "b3": _pack_vec(b3, CC),
    }

    ar512 = np.arange(512)
    in_maps = []
    for core in range(NCORES):
        b = core // 2
        own, others = _core_chunks(core)
        order = own + others
        xcat = np.concatenate([x[b, ch * 512:(ch + 1) * 512] for ch in order],
                              axis=0)                       # [2048, C]
        xT_p = np.ascontiguousarray(
            xcat.T.reshape(CC, P, TC).transpose(1, 0, 2))   # [P, CC, TC]
        kpos = np.concatenate([ch * 512 + ar512 for ch in order])
        qpos = np.concatenate([ch * 512 + ar512 for ch in own])
        mask = (kpos[:, None] <= qpos[None, :]).astype(BF16)      # [TC, TQ]
        mask16 = mask.reshape(16, P, TQ)
        # each key tile only ever sees one query half: lower (0:512) for
        # "full" units, upper (512:1024) for "half" units
        mask_p = np.empty((P, 16, 512), dtype=BF16)
        for st in range(16):
            q0 = 0 if st in FULL_ST else 512
            mask_p[:, st, :] = mask16[st, :, q0:q0 + 512]
        in_maps.append({**common,
                        "xq": np.ascontiguousarray(xT_p[:, :, 0:TQ]),
                        "xTb": xT_p.astype(BF16),
                        "maskT": mask_p})

    trace = bool(os.environ.get("BASS_TRACE"))
    res = run_bass_kernel_spmd(_PROG, in_maps, core_ids=list(range(NCORES)),
                               trace=trace)
    LAST_RESULT = res

    out = np.empty((B, T, C), dtype=np.float32)
    for core in range(NCORES):
        b = core // 2
        own, _ = _core_chunks(core)
        oT = res.results[core]["outT"]                      # [P, CC, TQ]
        o2 = oT.transpose(2, 1, 0).reshape(TQ, C)           # [q, C]
        for i, ch in enumerate(own):
            out[b, ch * 512:(ch + 1) * 512] = o2[i * 512:(i + 1) * 512]
    return out


# revision 75
# speedup vs baseline: 1.2076x; 1.1078x over previous
"""Trainium2 Bass kernel for a pre-LN transformer block (MHA + 3-layer FFN).

Sharding: data-parallel over (batch, query-chunk-pair) -> 8 cores, each
owning 1024 query tokens of one batch row.  To balance causal work, each
core owns a folded pair of 512-token chunks (chunk c and chunk 3-c of the
row), so both cores of a row see the same causal block structure.  K/V are
computed over the full 2048-token context per row (duplicated across the 2
cores sharing a row) - no cross-core collectives.  Context columns are
ordered own-chunks-first; the host-built per-core causal mask carries the
global positions.

On-device layout: all activations are TRANSPOSED, [feature, token],
features tiled 128-per-partition.  Every matmul is out^T = lhsT.T @ rhs
with lhsT = W in natural [in, out] layout, so the convention is maintained
end-to-end with zero on-chip transposes.  Attention scores are computed
transposed ([key, query]); softmax normalization reduces over the
partition (key) axis via a ones-column appended to V, so the denominator
falls out of the same PE accumulation that computes attn @ V.  LayerNorm
reduces over the partition (feature) axis with ones-vector matmuls
(float32r: single-pass fp32) on the PE.

Causal structure (uniform across cores by construction): s-tiles 0-7 are
the own chunks, 8-15 the other two chunks ascending.  Tiles {0-3, 8-11}
are needed by all 1024 queries ("full" units); tiles {4-7, 12-15} only by
the upper 512 queries ("half" units) - 25% of score/softmax/AV work is
statically skipped.
"""

import os

import numpy as np
import ml_dtypes

B, T, C = 4, 2048, 1024
H, D = 16, 64
FF = 4 * C
EPS = 1e-5
P = 128
CC = C // P          # 8 feature chunks
FOC = FF // P        # 32 ff chunks
TQ = 1024            # own (query) tokens per core
TC = 2048            # context tokens per core
NCORES = 8
SCALE = float(C) ** -0.5

FULL_ST = [0, 1, 2, 3, 8, 9, 10, 11]     # key tiles needed by all queries
HALF_ST = [4, 5, 6, 7, 12, 13, 14, 15]   # key tiles needed by queries 512:1024

BF16 = ml_dtypes.bfloat16

_PROG = None         # compiled Bacc program, built once per process
LAST_RESULT = None   # BassKernelResults of the most recent run (for profiling)


def _build_program():
    import concourse.mybir as mybir
    import concourse.tile as tile
    from concourse import bacc

    f32 = mybir.dt.float32
    f16 = mybir.dt.float16
    bf16 = mybir.dt.bfloat16
    AF = mybir.ActivationFunctionType
    ALU = mybir.AluOpType

    nc = bacc.Bacc("TRN2", target_bir_lowering=False, debug=False)

    # ---- DRAM parameters (per-core shapes; all pre-packed on host) ----
    xq = nc.declare_dram_parameter("xq", [P, CC, TQ], f32, isOutput=False)
    xTb = nc.declare_dram_parameter("xTb", [P, CC, TC], bf16, isOutput=False)
    maskT = nc.declare_dram_parameter("maskT", [P, 16, 512], bf16,
                                      isOutput=False)
    wq = nc.declare_dram_parameter("wq", [8, P, CC, P], bf16, isOutput=False)
    wk = nc.declare_dram_parameter("wk", [8, P, CC, P], bf16, isOutput=False)
    wv = nc.declare_dram_parameter("wv", [2, P, CC, 512], bf16, isOutput=False)
    wproj = nc.declare_dram_parameter("wproj", [P, CC, C], bf16, isOutput=False)
    w1 = nc.declare_dram_parameter("w1", [FOC, P, CC, P], bf16, isOutput=False)
    w2 = nc.declare_dram_parameter("w2", [FOC, P, FOC, P], bf16, isOutput=False)
    w3 = nc.declare_dram_parameter("w3", [CC, P, FOC, P], bf16, isOutput=False)
    bproj = nc.declare_dram_parameter("bproj", [P, CC], f32, isOutput=False)
    b1 = nc.declare_dram_parameter("b1", [P, FOC], f32, isOutput=False)
    b2 = nc.declare_dram_parameter("b2", [P, FOC], f32, isOutput=False)
    b3 = nc.declare_dram_parameter("b3", [P, CC], f32, isOutput=False)
    qb = nc.declare_dram_parameter("qb", [P, 8], f32, isOutput=False)
    kb = nc.declare_dram_parameter("kb", [P, 8], f32, isOutput=False)
    outT = nc.declare_dram_parameter("outT", [P, CC, TQ], f32, isOutput=True)

    TSW = 256  # layernorm column-slice width

    def layernorm_T(lp, lpr, lps, fill_xslice, ncols, out,
                    oc_col, ones_row, eps_sb):
        """Feature-axis LN (affine folded into downstream weights/biases on
        the host) of transposed bf16 activations, software-pipelined two
        slices deep: the normalize (PE broadcast + DVE) of slice ts-2 is
        emitted after the stats matmuls of slice ts so the in-order PE
        queue never stalls on the row-stats chain.  x and x^2 are packed
        adjacently per chunk so one bf16 ones-matmul (with 1/C folded into
        the ones value) yields both E[x] and E[x^2].  The reciprocal runs
        on the 128-lane broadcast of std, not the 1-lane row."""
        pends = []

        def emit_norm(p):
            ts, xsqs, mrow, std, sl = p
            ps_mu = lps.tile([P, TSW], f32, tag="ps_mu")
            nc.tensor.matmul(ps_mu[:], ones_row[:], mrow[:, 0:TSW],
                             start=True, stop=True)
            ps_rs = lps.tile([P, TSW], f32, tag="ps_rs")
            nc.tensor.matmul(ps_rs[:], ones_row[:], std[:],
                             start=True, stop=True)
            rsB = lp.tile([P, TSW], f32, tag="ln_rsB")
            nc.vector.reciprocal_approx_fast(rsB[:], ps_rs[:])
            for cc in range(CC):
                t1 = lp.tile([P, TSW], f32, tag="ln_t1")
                nc.vector.tensor_sub(t1[:], xsqs[cc][:, 0, :], ps_mu[:])
                nc.vector.tensor_mul(out[:, cc, sl], t1[:], rsB[:])

        for ts in range(ncols // TSW):
            sl = slice(ts * TSW, (ts + 1) * TSW)
            xsqs = [lp.tile([P, 2, TSW], bf16, tag=f"xsq{cc}", name=f"xsq{cc}")
                    for cc in range(CC)]
            fill_xslice(xsqs, ts, sl)
            ps_st = lps.tile([1, 2 * TSW], f32, tag="ps_st")
            for cc in range(CC):
                if cc < 4:
                    nc.scalar.square(xsqs[cc][:, 1, :], xsqs[cc][:, 0, :])
                else:
                    nc.vector.tensor_mul(xsqs[cc][:, 1, :], xsqs[cc][:, 0, :],
                                         xsqs[cc][:, 0, :])
                nc.tensor.matmul(ps_st[:], oc_col[:],
                                 xsqs[cc].rearrange("p a b -> p (a b)"),
                                 start=(cc == 0), stop=(cc == CC - 1))
            mrow = lpr.tile([1, 2 * TSW], f32, tag="ln_mrow")
            nc.vector.tensor_copy(mrow[:], ps_st[:])
            msq = lpr.tile([1, TSW], f32, tag="ln_msq")
            nc.vector.tensor_mul(msq[:], mrow[:, 0:TSW], mrow[:, 0:TSW])
            nc.vector.tensor_sub(mrow[:, TSW:2 * TSW],
                                 mrow[:, TSW:2 * TSW], msq[:])
            std = lpr.tile([1, TSW], f32, tag="ln_std")
            nc.scalar.activation(std[:], mrow[:, TSW:2 * TSW], AF.Sqrt,
                                 bias=eps_sb[:])
            pends.append((ts, xsqs, mrow, std, sl))
            if len(pends) > 2:
                emit_norm(pends.pop(0))
        for p in pends:
            emit_norm(p)

    with tile.TileContext(nc) as tc:
        with tc.tile_pool(name="const", bufs=1) as cp:
            oc_col = cp.tile([P, 1], bf16)
            nc.vector.memset(oc_col[:], 1.0 / C)
            ones_row = cp.tile([1, P], f32)
            nc.vector.memset(ones_row[:], 1.0)
            eps_sb = cp.tile([1, 1], f32)
            nc.vector.memset(eps_sb[:], EPS)
            ones_row_h = cp.tile([1, 64], f16)
            nc.vector.memset(ones_row_h[:], 1.0)

            qb_sb = cp.tile([P, 8], f32, tag="qb")
            nc.sync.dma_start(qb_sb[:], qb[:])
            kb_sb = cp.tile([P, 8], f32, tag="kb")
            nc.sync.dma_start(kb_sb[:], kb[:])
            bproj_sb = cp.tile([P, CC], f32, tag="bproj")
            nc.sync.dma_start(bproj_sb[:], bproj[:])
            b1_sb = cp.tile([P, FOC], f32, tag="b1")
            nc.sync.dma_start(b1_sb[:], b1[:])
            b2_sb = cp.tile([P, FOC], f32, tag="b2")
            nc.sync.dma_start(b2_sb[:], b2[:])
            b3_sb = cp.tile([P, CC], f32, tag="b3")
            nc.sync.dma_start(b3_sb[:], b3[:])

            with tc.tile_pool(name="x1", bufs=1) as p_x1:
                x1T = p_x1.tile([P, CC, TQ], f32, tag="x1T")

                # ======== attention residual branch ========
                with tc.tile_pool(name="ao", bufs=1) as p_ao:
                    attnT = p_ao.tile([P, CC, TQ], bf16, tag="attnT")

                    with tc.tile_pool(name="qkv", bufs=1) as qp:
                        kT = qp.tile([P, 8, TC], bf16, tag="kT")   # head pairs
                        qT = qp.tile([P, 8, TQ], bf16, tag="qT")
                        V_aug = qp.tile([P, 16, H, 65], bf16, tag="V_aug")
                        nc.vector.memset(V_aug[:], 1.0)  # ones col @ index 64

                        with tc.tile_pool(name="h", bufs=1) as php:
                            hT = php.tile([P, CC, TC], bf16, tag="hT")

                            # ---------------- LN1 over full context --------
                            with (
                                tc.tile_pool(name="ln1p", bufs=3) as lp,
                                tc.tile_pool(name="ln1r", bufs=3) as lpr,
                                tc.tile_pool(name="ln1ps", bufs=2,
                                             space="PSUM") as lps,
                            ):
                                def fill_x1slice(xsqs, ts, sl):
                                    for cc in range(CC):
                                        nc.sync.dma_start(xsqs[cc][:, 0, :],
                                                          xTb[:, cc, sl])
                                layernorm_T(lp, lpr, lps, fill_x1slice, TC,
                                            hT, oc_col, ones_row, eps_sb)

                            # ------- merged QKV + attention -------
                            # QKV work is cut into small self-contained
                            # matmul chains.  Pairs 0-1 and the first half of
                            # V run up front; the rest is drip-fed between
                            # attention units so the in-order PE queue never
                            # drains (keeps the HAM clock warm).
                            half_pairs = [(HALF_ST[i], HALF_ST[i + 1])
                                          for i in range(0, 8, 2)]
                            with (
                                tc.tile_pool(name="qkvw", bufs=2) as qwp,
                                tc.tile_pool(name="qkvps", bufs=2,
                                             space="PSUM") as qps,
                                tc.tile_pool(name="attn", bufs=2) as atp,
                                tc.tile_pool(name="attnm", bufs=1) as amp,
                                tc.tile_pool(name="attnps", bufs=2,
                                             space="PSUM") as aps,
                                tc.tile_pool(name="attnps1", bufs=1,
                                             space="PSUM") as aps1,
                            ):
                                mask_sb = amp.tile([P, 16, 512], bf16,
                                                   tag="mask")
                                nc.sync.dma_start(mask_sb[:], maskT[:])

                                # Each chain is split (load, run): the weight
                                # DMA is issued two chains ahead of its
                                # matmuls so the PE never waits on a load.
                                def kq_chain(wsrc, bias_sb, dest_kT, pair, js):
                                    sl = slice(js * 512, (js + 1) * 512)

                                    def load():
                                        wt = qwp.tile([P, CC, P], bf16,
                                                      tag="wkq", name="wkq")
                                        nc.sync.dma_start(wt[:], wsrc[pair])

                                        def run():
                                            ps_f = qps.tile([P, 512], f32,
                                                            tag="ps_f",
                                                            name="ps_f")
                                            for cc in range(CC):
                                                nc.tensor.matmul(
                                                    ps_f[:], wt[:, cc, :],
                                                    hT[:, cc, sl],
                                                    start=(cc == 0),
                                                    stop=(cc == CC - 1))
                                            nc.vector.tensor_scalar_add(
                                                dest_kT[:, pair, sl], ps_f[:],
                                                bias_sb[:, pair:pair + 1])
                                        return run
                                    return load

                                def pair_chains(pair):
                                    return ([kq_chain(wk, kb_sb, kT, pair, js)
                                             for js in range(4)]
                                            + [kq_chain(wq, qb_sb, qT, pair, js)
                                               for js in range(2)])

                                loaded = []

                                def drip(n):
                                    for _ in range(n):
                                        if fill:
                                            loaded.append(fill.pop()())
                                        if len(loaded) > 2:
                                            loaded.pop(0)()

                                def drain():
                                    while loaded:
                                        loaded.pop(0)()

                                # up-front: pairs 0-1, then all of V with a
                                # wide moving operand (amortizes LDWEIGHTS;
                                # the scoped weight pool frees its SBUF
                                # before the attention working tiles land)
                                fill = pair_chains(0) + pair_chains(1)
                                fill = list(reversed(fill))
                                drip(len(fill))
                                drain()
                                with tc.tile_pool(name="vw", bufs=1) as vwp:
                                    for js2 in range(2):
                                        wv_t = vwp.tile([P, CC, 512], bf16,
                                                        tag="wv_t")
                                        nc.sync.dma_start(wv_t[:], wv[js2])
                                        for sc in range(16):
                                            ps_f = qps.tile([P, 512], f32,
                                                            tag="ps_f",
                                                            name="ps_f")
                                            for cc in range(CC):
                                                nc.tensor.matmul(
                                                    ps_f[:],
                                                    hT[:, cc,
                                                       sc * P:(sc + 1) * P],
                                                    wv_t[:, cc, :],
                                                    start=(cc == 0),
                                                    stop=(cc == CC - 1))
                                            for hh in range(8):
                                                hd = js2 * 8 + hh
                                                nc.vector.tensor_copy(
                                                    V_aug[:, sc, hd, 0:64],
                                                    ps_f[:,
                                                         hh * 64:(hh + 1) * 64])

                                # drip-fed fillers with readiness deadlines:
                                # (chains-remaining ceiling before head h)
                                fill = []
                                for pr in range(2, 8):
                                    fill += pair_chains(pr)
                                n_fill = len(fill)   # 36
                                deadline = {4: 30, 6: 24, 8: 18,
                                            10: 12, 12: 6, 14: 0}
                                fill = list(reversed(fill))  # pop() from end

                                def pump(target_remaining):
                                    # run chains until at most
                                    # target_remaining are not yet executed
                                    while len(fill) + len(loaded) > \
                                            target_remaining:
                                        if fill and len(loaded) <= 2:
                                            loaded.append(fill.pop()())
                                        elif loaded:
                                            loaded.pop(0)()

                                pend = None

                                def emit_norm(p):
                                    dn, avf, h = p
                                    pair, off = h // 2, (h % 2) * 64
                                    for js in range(2):
                                        sl = slice(js * 512, (js + 1) * 512)
                                        ps_bc = qps.tile([P, 512], f32,
                                                         tag="ps_f",
                                                         name="ps_f")
                                        nc.tensor.matmul(ps_bc[0:64, :],
                                                         ones_row_h[:],
                                                         dn[:, sl],
                                                         start=True, stop=True)
                                        rcB = atp.tile([64, 512], f32,
                                                       tag="rcB", name="rcB")
                                        nc.vector.reciprocal_approx_fast(
                                            rcB[:], ps_bc[0:64, :])
                                        nc.vector.tensor_mul(
                                            attnT[off:off + 64, pair, sl],
                                            avf[:, sl], rcB[:])

                                for h in range(H):
                                    pair, off = h // 2, (h % 2) * 64
                                    if h in deadline:
                                        pump(deadline[h])
                                    ps_av = aps1.tile([65, TQ], f32,
                                                      tag="ps_av")
                                    # 8 full units: all 1024 queries vs one
                                    # key tile
                                    for ui, st in enumerate(FULL_ST):
                                        if ui == 2 and pend is not None:
                                            emit_norm(pend)
                                            pend = None
                                        if ui in (1, 5):
                                            drip(1)
                                        ps_s = aps.tile([P, TQ], f32,
                                                        tag="ps_s")
                                        for js in range(2):
                                            sl = slice(js * 512, (js + 1) * 512)
                                            nc.tensor.matmul(
                                                ps_s[:, sl],
                                                kT[off:off + 64, pair,
                                                   st * P:(st + 1) * P],
                                                qT[off:off + 64, pair, sl],
                                                start=True, stop=True)
                                        ex = atp.tile([P, TQ], bf16, tag="ex")
                                        nc.scalar.activation(ex[:], ps_s[:],
                                                             AF.Exp,
                                                             scale=SCALE)
                                        # only queries 0:512 can be masked
                                        # here; the upper half is statically
                                        # all-ones
                                        nc.vector.tensor_mul(
                                            ex[:, 0:512], ex[:, 0:512],
                                            mask_sb[:, st, :])
                                        for js in range(2):
                                            sl = slice(js * 512, (js + 1) * 512)
                                            nc.tensor.matmul(
                                                ps_av[:, sl],
                                                V_aug[:, st, h, :],
                                                ex[:, sl],
                                                start=(ui == 0),
                                                stop=(ui == 7 and js == 0))
                                    # 4 half-unit pairs: queries 512:1024 vs
                                    # two key tiles, sharing one psum/exp
                                    for pi, (stA, stB) in enumerate(half_pairs):
                                        if pi == 1 and h < 8:
                                            drip(1)
                                        ps_s = aps.tile([P, TQ], f32,
                                                        tag="ps_s")
                                        for js, st in ((0, stA), (1, stB)):
                                            sl = slice(js * 512, (js + 1) * 512)
                                            nc.tensor.matmul(
                                                ps_s[:, sl],
                                                kT[off:off + 64, pair,
                                                   st * P:(st + 1) * P],
                                                qT[off:off + 64, pair,
                                                   512:1024],
                                                start=True, stop=True)
                                        ex = atp.tile([P, TQ], bf16, tag="ex")
                                        nc.scalar.activation(ex[:], ps_s[:],
                                                             AF.Exp,
                                                             scale=SCALE)
                                        for js, st in ((0, stA), (1, stB)):
                                            sl = slice(js * 512, (js + 1) * 512)
                                            nc.vector.tensor_mul(
                                                ex[:, sl], ex[:, sl],
                                                mask_sb[:, st, :])
                                            nc.tensor.matmul(
                                                ps_av[:, 512:1024],
                                                V_aug[:, st, h, :], ex[:, sl],
                                                start=False,
                                                stop=(pi == 3 and js == 1))
                                    dn = atp.tile([1, TQ], f16, tag="dn")
                                    nc.vector.tensor_copy(dn[:],
                                                          ps_av[64:65, :])
                                    avf = atp.tile([64, TQ], bf16, tag="avf")
                                    nc.vector.tensor_copy(avf[:],
                                                          ps_av[0:64, :])
                                    pend = (dn, avf, h)
                                pump(0)
                                emit_norm(pend)

                    # ---------------- out-projection + residual ------------
                    with (
                        tc.tile_pool(name="proj", bufs=3) as pp,
                        tc.tile_pool(name="projw", bufs=1) as pwp,
                        tc.tile_pool(name="projps", bufs=4, space="PSUM") as pps,
                    ):
                        wp_sb = pwp.tile([P, CC, C], bf16, tag="wproj")
                        nc.sync.dma_start(wp_sb[:], wproj[:])
                        xown = pwp.tile([P, CC, TQ], f32, tag="xown")
                        nc.sync.dma_start(xown[:], xq[:])
                        for co in range(CC):
                            for js in range(2):
                                sl = slice(js * 512, (js + 1) * 512)
                                ps_p = pps.tile([P, 512], f32, tag="ps_p")
                                for ci in range(CC):
                                    nc.tensor.matmul(
                                        ps_p[:],
                                        wp_sb[:, ci, co * P:(co + 1) * P],
                                        attnT[:, ci, sl],
                                        start=(ci == 0), stop=(ci == CC - 1))
                                t1 = pp.tile([P, 512], f32, tag="pj_t1")
                                nc.vector.tensor_scalar_add(
                                    t1[:], ps_p[:], bproj_sb[:, co:co + 1])
                                nc.vector.tensor_add(x1T[:, co, sl], t1[:],
                                                     xown[:, co, sl])

                # ======== FFN residual branch ========
                with tc.tile_pool(name="h2", bufs=1) as p_h2:
                    h2T = p_h2.tile([P, FOC, TQ], bf16, tag="h2T")

                    with tc.tile_pool(name="f1", bufs=1) as fp_in:
                        h2in = fp_in.tile([P, CC, TQ], bf16, tag="h2in")
                        # -------- LN2 --------
                        with (
                            tc.tile_pool(name="ln2p", bufs=3) as lp2,
                            tc.tile_pool(name="ln2r", bufs=3) as lpr2,
                            tc.tile_pool(name="ln2ps", bufs=2,
                                         space="PSUM") as lps2,
                        ):
                            def fill_x2slice(xsqs, ts, sl):
                                for cc in range(CC):
                                    nc.vector.tensor_copy(xsqs[cc][:, 0, :],
                                                          x1T[:, cc, sl])
                            layernorm_T(lp2, lpr2, lps2, fill_x2slice, TQ,
                                        h2in, oc_col, ones_row, eps_sb)
                        # -------- Y1 = relu(h2in @ W1 + b1) --------
                        with (
                            tc.tile_pool(name="f1w", bufs=2) as fwp1,
                            tc.tile_pool(name="f1ps", bufs=4,
                                         space="PSUM") as fps1,
                        ):
                            for fo in range(FOC):
                                w1t = fwp1.tile([P, CC, P], bf16, tag="w1t")
                                nc.sync.dma_start(w1t[:], w1[fo])
                                for js in range(2):
                                    sl = slice(js * 512, (js + 1) * 512)
                                    ps1 = fps1.tile([P, 512], f32, tag="ps1")
                                    for cc in range(CC):
                                        nc.tensor.matmul(
                                            ps1[:], w1t[:, cc, :],
                                            h2in[:, cc, sl],
                                            start=(cc == 0), stop=(cc == CC - 1))
                                    nc.scalar.activation(
                                        h2T[:, fo, sl], ps1[:], AF.Relu,
                                        bias=b1_sb[:, fo:fo + 1])

                    # -------- Y2 = relu(h2T @ W2 + b2) --------
                    with tc.tile_pool(name="h3", bufs=1) as p_h3:
                        h3T = p_h3.tile([P, FOC, TQ], bf16, tag="h3T")
                        with (
                            tc.tile_pool(name="f2w", bufs=3) as fwp2,
                            tc.tile_pool(name="f2ps", bufs=4,
                                         space="PSUM") as fps2,
                        ):
                            for fo in range(FOC):
                                w2t = fwp2.tile([P, FOC, P], bf16, tag="w2t")
                                nc.sync.dma_start(w2t[:], w2[fo])
                                for js in range(2):
                                    sl = slice(js * 512, (js + 1) * 512)
                                    ps2 = fps2.tile([P, 512], f32, tag="ps2")
                                    for fi in range(FOC):
                                        nc.tensor.matmul(
                                            ps2[:], w2t[:, fi, :],
                                            h2T[:, fi, sl],
                                            start=(fi == 0),
                                            stop=(fi == FOC - 1))
                                    nc.scalar.activation(
                                        h3T[:, fo, sl], ps2[:], AF.Relu,
                                        bias=b2_sb[:, fo:fo + 1])

                        # -------- Y3 + bias + residual -> out --------
                        with (
                            tc.tile_pool(name="f3", bufs=3) as fp3,
                            tc.tile_pool(name="f3w", bufs=2) as fwp3,
                            tc.tile_pool(name="f3ps", bufs=4,
                                         space="PSUM") as fps3,
                        ):
                            for co in range(CC):
                                w3t = fwp3.tile([P, FOC, P], bf16, tag="w3t")
                                nc.sync.dma_start(w3t[:], w3[co])
                                for js in range(2):
                                    sl = slice(js * 512, (js + 1) * 512)
                                    ps3 = fps3.tile([P, 512], f32, tag="ps3")
                                    for fi in range(FOC):
                                        nc.tensor.matmul(
                                            ps3[:], w3t[:, fi, :],
                                            h3T[:, fi, sl],
                                            start=(fi == 0),
                                            stop=(fi == FOC - 1))
                                    ot = fp3.tile([P, 512], f32, tag="ot")
                                    nc.vector.tensor_scalar_add(
                                        ot[:], ps3[:], b3_sb[:, co:co + 1])
                                    nc.vector.tensor_add(ot[:], ot[:],
                                                         x1T[:, co, sl])
                                    nc.sync.dma_start(outT[:, co, sl], ot[:])

    nc.compile()
    return nc


def _pack_vec(v, nchunks):
    # [nchunks*P] -> [P, nchunks]
    return np.ascontiguousarray(np.asarray(v, dtype=np.float32).reshape(nchunks, P).T)


def _pack_w(w, in_chunks, out_dim):
    # [in, out] -> [P, in_chunks, out]  (lhsT tiles, fully resident)
    w = np.asarray(w).astype(BF16)
    return np.ascontiguousarray(w.reshape(in_chunks, P, out_dim).transpose(1, 0, 2))


def _pack_w_stream(w, in_chunks, out_chunks, out_w=P):
    # [in, out] -> [out_chunks, P, in_chunks, out_w]  (streamed lhsT tiles)
    w = np.asarray(w).astype(BF16)
    return np.ascontiguousarray(
        w.reshape(in_chunks, P, out_chunks, out_w).transpose(2, 1, 0, 3))


def _core_chunks(core):
    """Global 512-token chunk ids (of the batch row) in this core's context
    column order: own folded pair first, then the other two ascending."""
    c = core % 2
    own = [c, 3 - c]
    others = sorted(set(range(4)) - set(own))
    return own, others


def kernel(x, Wq, Wk, Wv, Wproj, bproj, W1, b1, W2, b2, W3, b3,
           ln1_g, ln1_b, ln2_g, ln2_b):
    global _PROG, LAST_RESULT
    from concourse.bass_utils import run_bass_kernel_spmd

    if _PROG is None:
        _PROG = _build_program()

    x = np.asarray(x, dtype=np.float32)

    # Fold the LN affine transforms into the consuming weights/biases:
    #   h_full = g * h_raw + b  with  h_raw = (x - mu) * rstd
    # Q/K/V/W1 rows get scaled by g; the b contribution becomes an additive
    # bias: per-d for Q/K (applied at PSUM eviction), constant-through-
    # softmax for V (folded into bproj via Wproj), and per-ff for W1.
    g1 = np.asarray(ln1_g, dtype=np.float32)[:, None]
    b1v = np.asarray(ln1_b, dtype=np.float32)
    g2 = np.asarray(ln2_g, dtype=np.float32)[:, None]
    b2v = np.asarray(ln2_b, dtype=np.float32)
    Wq2 = np.asarray(Wq, dtype=np.float32).transpose(1, 0, 2).reshape(C, C)
    Wk2 = np.asarray(Wk, dtype=np.float32).transpose(1, 0, 2).reshape(C, C)
    Wv2 = np.asarray(Wv, dtype=np.float32).transpose(1, 0, 2).reshape(C, C)
    W1f = np.asarray(W1, dtype=np.float32)
    vb = b1v @ Wv2                                   # [C], per-(h,d) V bias
    bproj_f = np.asarray(bproj, dtype=np.float32) + vb @ np.asarray(
        Wproj, dtype=np.float32)
    b1_f = np.asarray(b1, dtype=np.float32) + b2v @ W1f

    common = {
        "wq": _pack_w_stream(g1 * Wq2, CC, 8),
        "wk": _pack_w_stream(g1 * Wk2, CC, 8),
        "wv": _pack_w_stream(g1 * Wv2, CC, 2, 512),
        "wproj": _pack_w(Wproj, CC, C),
        "w1": _pack_w_stream(g2 * W1f, CC, FOC),
        "w2": _pack_w_stream(W2, FOC, FOC),
        "w3": _pack_w_stream(W3, FOC, CC),
        "qb": _pack_vec(b1v @ Wq2, 8),
        "kb": _pack_vec(b1v @ Wk2, 8),
        "bproj": _pack_vec(bproj_f, CC),
        "b1": _pack_vec(b1_f, FOC), "b2": _pack_vec(b2, FOC),
        "b3": _pack_vec(b3, CC),
    }

    ar512 = np.arange(512)
    in_maps = []
    for core in range(NCORES):
        b = core // 2
        own, others = _core_chunks(core)
        order = own + others
        xcat = np.concatenate([x[b, ch * 512:(ch + 1) * 512] for ch in order],
                              axis=0)                       # [2048, C]
        xT_p = np.ascontiguousarray(
            xcat.T.reshape(CC, P, TC).transpose(1, 0, 2))   # [P, CC, TC]
        kpos = np.concatenate([ch * 512 + ar512 for ch in order])
        qpos = np.concatenate([ch * 512 + ar512 for ch in own])
        mask = (kpos[:, None] <= qpos[None, :]).astype(BF16)      # [TC, TQ]
        mask16 = mask.reshape(16, P, TQ)
        # each key tile only ever sees one query half: lower (0:512) for
        # "full" units, upper (512:1024) for "half" units
        mask_p = np.empty((P, 16, 512), dtype=BF16)
        for st in range(16):
            q0 = 0 if st in FULL_ST else 512
            mask_p[:, st, :] = mask16[st, :, q0:q0 + 512]
        in_maps.append({**common,
                        "xq": np.ascontiguousarray(xT_p[:, :, 0:TQ]),
                        "xTb": xT_p.astype(BF16),
                        "maskT": mask_p})

    trace = bool(os.environ.get("BASS_TRACE"))
    res = run_bass_kernel_spmd(_PROG, in_maps, core_ids=list(range(NCORES)),
                               trace=trace)
    LAST_RESULT = res

    out = np.empty((B, T, C), dtype=np.float32)
    for core in range(NCORES):
        b = core // 2
        own, _ = _core_chunks(core)
        oT = res.results[core]["outT"]                      # [P, CC, TQ]
        o2 = oT.transpose(2, 1, 0).reshape(TQ, C)           # [q, C]
        for i, ch in enumerate(own):
            out[b, ch * 512:(ch + 1) * 512] = o2[i * 512:(i + 1) * 512]
    return out
